# revision 1
# baseline (speedup 1.0000x reference)
# Bass/Trainium2 kernel for nn_L2PairwiceObjectiveFunction (pairwise L2 loss
# between per-row linear interpolations of two curve sets onto a common
# uniform grid).
#
# Full inputs: x, y1, y2 [1024, 8192] f32 (x sorted per row).
# Output: [1024, 1024] f32.
#
# Sharding: batch rows split across 8 NeuronCores (128 rows each, rows on
# SBUF partitions). The pairwise bilinear form uses an AllGather of the
# transposed [3072, 128] interpolated y2 grids (bf16) followed by a local
# PE matmul.
#
# Interpolation algorithm (searchsorted-free): the common grid is UNIFORM,
# so each data point's grid cell is computable elementwise:
# c[n] = floor((x[n]-xmin)/dx) + 1, clipped to [0, 3000]. For grid point m
# the bracketing segment is the last n with c[n] <= m. We scatter per-datum
# quantities (cell marker, frac(x), gap, y-lo, y-next; int16-quantized)
# into grid bins with gpsimd local_scatter (true per-partition indices;
# last-datum-per-bin enforced by a dedup mask so indices are unique), then
# fill empty bins with a carry-forward tensor_tensor_scan
# (state = empty*state + value). Interpolation is then pure elementwise
# work. Bin space is processed in two scatter halves x two scan/interp
# quarters to fit SBUF.

import numpy as np

B, N, M, NCORES = 1024, 8192, 3000, 8
R = B // NCORES  # 128 rows per core
P = 128
NBINS = 3004        # 2*HBINS bins (c clipped to [0, 3000])
HBINS = 1502        # bins per scatter half: [0,1502), [1502,3004)
QBINS = 751         # bins per scan/interp quarter
NIDX = 4608         # datum window per half (covers Binomial spread at ~11 sigma)
WOFF = (0, N - NIDX)   # window starts per half
WPAD = NIDX + 16    # padded quant-tile width (need NIDX+1 for shifted reads)
USCALE = 32766.0
YSCALE = 3000.0
DXSCALE = 1e7
DXCLIP = 3.2e-3
KT = 24             # matmul k-tiles; grid padded 3000 -> 3072
MT = KT * P
WB = 512            # stage-A column block


def build_nc(xmin, xmax, debug=False):
    import concourse.bacc as bacc
    import concourse.mybir as mybir
    from concourse.tile import TileContext
    from concourse import library_config
    from concourse.tile_rust import add_dep_helper

    F32, BF16, I16 = mybir.dt.float32, mybir.dt.bfloat16, mybir.dt.int16
    A = mybir.AluOpType
    AF = mybir.ActivationFunctionType

    dx = float((np.float32(xmax) - np.float32(xmin)) / np.float32(M - 1))
    inv_dx = float(np.float32(1.0) / np.float32(dx))

    nc = bacc.Bacc("TRN2", target_bir_lowering=False)
    x_in = nc.dram_tensor("x", [R, N], F32, kind="ExternalInput")
    y1_in = nc.dram_tensor("y1", [R, N], F32, kind="ExternalInput")
    y2_in = nc.dram_tensor("y2", [R, N], F32, kind="ExternalInput")
    xc_in = nc.dram_tensor("xc", [1, M], F32, kind="ExternalInput")
    id_in = nc.dram_tensor("ident", [P, P], BF16, kind="ExternalInput")
    o_out = nc.dram_tensor("out", [R, B], F32, kind="ExternalOutput")
    dbg = {}
    if debug:
        for nm, w, dt in [
            ("d_cp1", WPAD, I16), ("d_u16", WPAD, I16), ("d_dxq", WPAD, I16),
            ("d_y1q", WPAD, I16), ("d_idx", NIDX, I16),
            ("d_fcp1", NBINS, F32), ("d_fu", NBINS, F32), ("d_fdx", NBINS, F32),
            ("d_fy1", NBINS, F32), ("d_fy1n", NBINS, F32),
            ("d_y1c", M, BF16), ("d_y2c", M, BF16), ("d_sq1", 1, F32),
        ]:
            dbg[nm] = nc.dram_tensor(nm, [R, w], dt, kind="ExternalOutput")

    with TileContext(nc) as tc:
        with (
            tc.tile_pool(name="pers", bufs=1) as pers,
            tc.tile_pool(name="psum", bufs=2, space="PSUM") as pp,
            tc.tile_pool(name="mmpsum", bufs=1, space="PSUM") as mmpp,
            tc.tile_pool(name="dram", bufs=1, space="DRAM") as dp,
        ):
            lib_bi = nc.gpsimd.load_library(library_config.local_scatter)

            x0 = pers.tile([P, 1], F32, tag="x0")
            xlast = pers.tile([P, 1], F32, tag="xlast")
            nc.sync.dma_start(out=x0[:], in_=x_in[:, 0:1])
            nc.sync.dma_start(out=xlast[:], in_=x_in[:, N - 1:N])
            negone = pers.tile([P, 1], I16, tag="negone")
            nc.vector.memset(negone[:], -1)

            y1c = pers.tile([P, MT], BF16, tag="y1c")
            y2c = pers.tile([P, MT], BF16, tag="y2c")
            nc.vector.memset(y1c[:, M:], 0)
            nc.vector.memset(y2c[:, M:], 0)
            sqacc = {}
            for ynm in ("y1", "y2"):
                s = pers.tile([P, 1], F32, tag=f"sqacc_{ynm}")
                nc.vector.memset(s[:], 0)
                sqacc[ynm] = s
            carries = {}   # latest scan carry [P,1] per array
            inits = {}     # scan initials from first datum
            ANAMES = ("cp1", "u", "dx", "y1", "y1n", "y2", "y2n")

            with (
                tc.tile_pool(name="qp", bufs=1) as qp,
                tc.tile_pool(name="sp", bufs=2) as sp,
            ):
                for h in range(2):
                    woff = WOFF[h]
                    # ---- stage A: quantize datum window [woff, woff+NIDX] ----
                    cp1 = qp.tile([P, WPAD], I16, tag="cp1")
                    u16 = qp.tile([P, WPAD], I16, tag="u16")
                    dxq = qp.tile([P, WPAD], I16, tag="dxq")
                    y1q = qp.tile([P, WPAD], I16, tag="y1q")
                    y2q = qp.tile([P, WPAD], I16, tag="y2q")
                    for t in (cp1, u16, dxq, y1q, y2q):
                        nc.vector.memset(t[:, NIDX:], 0)
                    for bi in range(NIDX // WB):
                        lo = woff + bi * WB
                        wext = WB + 1 if lo + WB < N else WB
                        sl = slice(bi * WB, bi * WB + WB)
                        xb = sp.tile([P, WB + 1], F32, tag="xb")
                        nc.sync.dma_start(out=xb[:, :wext],
                                          in_=x_in[:, lo:lo + wext])
                        if wext == WB:
                            nc.vector.memset(xb[:, WB:], 0)
                        # t5 = (x - xmin)/dx + 0.5, clipped to [-0.5, 3000.0]
                        t5 = sp.tile([P, WB], F32, tag="t5")
                        nc.scalar.activation(t5[:], xb[:, :WB], AF.Copy,
                                             bias=float(0.5 - xmin * inv_dx),
                                             scale=inv_dx)
                        nc.vector.tensor_scalar(out=t5[:], in0=t5[:],
                                                scalar1=3000.0, scalar2=-0.5,
                                                op0=A.min, op1=A.max)
                        # c16 = round(t5) = floor(t)+1
                        c16b = sp.tile([P, WB], I16, tag="c16b")
                        nc.vector.tensor_copy(out=c16b[:], in_=t5[:])
                        nc.vector.tensor_scalar(out=cp1[:, sl], in0=c16b[:],
                                                scalar1=1, scalar2=None, op0=A.add)
                        # u16 = round((t5 + 0.5 - c16) * USCALE)
                        cf = sp.tile([P, WB], F32, tag="cf")
                        nc.scalar.copy(out=cf[:], in_=c16b[:])
                        nc.vector.scalar_tensor_tensor(out=t5[:], in0=t5[:],
                                                       scalar=0.5, in1=cf[:],
                                                       op0=A.add, op1=A.subtract)
                        nc.scalar.activation(u16[:, sl], t5[:], AF.Copy,
                                             scale=USCALE)
                        # gap -> dxq
                        xd = sp.tile([P, WB], F32, tag="xd")
                        nc.vector.tensor_tensor(out=xd[:], in0=xb[:, 1:WB + 1],
                                                in1=xb[:, :WB], op=A.subtract)
                        nc.vector.tensor_scalar(out=dxq[:, sl], in0=xd[:],
                                                scalar1=DXCLIP, scalar2=DXSCALE,
                                                op0=A.min, op1=A.mult)
                        # y quantization
                        yb = sp.tile([P, WB], F32, tag="yb")
                        nc.sync.dma_start(out=yb[:], in_=y1_in[:, lo:lo + WB])
                        nc.scalar.activation(y1q[:, sl], yb[:], AF.Copy,
                                             scale=YSCALE)
                        yb2 = sp.tile([P, WB], F32, tag="yb")
                        nc.sync.dma_start(out=yb2[:], in_=y2_in[:, lo:lo + WB])
                        nc.scalar.activation(y2q[:, sl], yb2[:], AF.Copy,
                                             scale=YSCALE)
                    if h == 0:
                        # col NIDX (shifted reads): quantize datum NIDX
                        xe = sp.tile([P, 4], F32, tag="xe")
                        nc.sync.dma_start(out=xe[:, 0:1], in_=x_in[:, NIDX:NIDX + 1])
                        nc.sync.dma_start(out=xe[:, 1:2], in_=y1_in[:, NIDX:NIDX + 1])
                        nc.sync.dma_start(out=xe[:, 2:3], in_=y2_in[:, NIDX:NIDX + 1])
                        t5e = sp.tile([P, 1], F32, tag="t5e")
                        nc.scalar.activation(t5e[:], xe[:, 0:1], AF.Copy,
                                             bias=float(0.5 - xmin * inv_dx),
                                             scale=inv_dx)
                        nc.vector.tensor_scalar(out=t5e[:], in0=t5e[:],
                                                scalar1=3000.0, scalar2=-0.5,
                                                op0=A.min, op1=A.max)
                        c16e = sp.tile([P, 1], I16, tag="c16e")
                        nc.vector.tensor_copy(out=c16e[:], in_=t5e[:])
                        nc.vector.tensor_scalar(out=cp1[:, NIDX:NIDX + 1],
                                                in0=c16e[:], scalar1=1,
                                                scalar2=None, op0=A.add)
                        nc.scalar.activation(y1q[:, NIDX:NIDX + 1], xe[:, 1:2],
                                             AF.Copy, scale=YSCALE)
                        nc.scalar.activation(y2q[:, NIDX:NIDX + 1], xe[:, 2:3],
                                             AF.Copy, scale=YSCALE)
                        # scan initials from datum 0
                        for nm, src in [("cp1", cp1[:, 0:1]), ("u", u16[:, 0:1]),
                                        ("y1", y1q[:, 0:1]), ("y1n", y1q[:, 1:2]),
                                        ("y2", y2q[:, 0:1]), ("y2n", y2q[:, 1:2])]:
                            it = pers.tile([P, 1], F32, tag=f"init_{nm}")
                            nc.vector.tensor_copy(out=it[:], in_=src)
                            inits[nm] = it
                        inits["dx"] = 0.0

                    # ---- dedup + bin-index mask --------------------------
                    neq = qp.tile([P, NIDX], I16, tag="neq")
                    nc.vector.tensor_tensor(out=neq[:], in0=cp1[:, 0:NIDX],
                                            in1=cp1[:, 1:NIDX + 1], op=A.not_equal)
                    if h == 1:
                        nc.vector.memset(neq[:, NIDX - 1:], 0)
                    idx = qp.tile([P, NIDX], I16, tag="idx")
                    nc.vector.memset(idx[:], 0)
                    nc.vector.copy_predicated(out=idx[:], mask=neq[:],
                                              data=cp1[:, 0:NIDX])
                    nc.vector.tensor_scalar(out=idx[:], in0=idx[:], scalar1=1,
                                            scalar2=None, op0=A.subtract)
                    sel = qp.tile([P, NIDX], I16, tag="neq")  # reuse slot
                    if h == 0:
                        nc.vector.tensor_scalar(out=sel[:], in0=idx[:],
                                                scalar1=HBINS - 1, scalar2=None,
                                                op0=A.is_gt)
                        nc.vector.copy_predicated(
                            out=idx[:], mask=sel[:],
                            data=negone[:].to_broadcast([P, NIDX]))
                    else:
                        nc.vector.tensor_scalar(out=sel[:], in0=idx[:],
                                                scalar1=HBINS - 1, scalar2=None,
                                                op0=A.is_le)
                        nc.vector.tensor_scalar(out=idx[:], in0=idx[:],
                                                scalar1=HBINS, scalar2=None,
                                                op0=A.subtract)
                        nc.vector.copy_predicated(
                            out=idx[:], mask=sel[:],
                            data=negone[:].to_broadcast([P, NIDX]))

                    if debug and h == 0:
                        for nm, t in [("d_cp1", cp1), ("d_u16", u16),
                                      ("d_dxq", dxq), ("d_y1q", y1q),
                                      ("d_idx", idx)]:
                            nc.sync.dma_start(out=dbg[nm][:], in_=t[:])

                    # ---- scatters (7 arrays into this half's bins) -------
                    # local_scatter mishandles data APs with a nonzero offset
                    # (drops some writes), so the "next-datum" arrays are
                    # scattered with a materialized shifted INDEX array
                    # instead: value y[j] goes to the bin of datum j-1.
                    idxp = qp.tile([P, NIDX], I16, tag="idxp")
                    nc.vector.memset(idxp[:, 0:1], -1)
                    nc.vector.tensor_copy(out=idxp[:, 1:NIDX],
                                          in_=idx[:, 0:NIDX - 1])
                    adata = {
                        "cp1": (cp1[:, 0:NIDX], idx), "u": (u16[:, 0:NIDX], idx),
                        "dx": (dxq[:, 0:NIDX], idx),
                        "y1": (y1q[:, 0:NIDX], idx),
                        "y1n": (y1q[:, 0:NIDX], idxp),
                        "y2": (y2q[:, 0:NIDX], idx),
                        "y2n": (y2q[:, 0:NIDX], idxp),
                    }
                    dsts = {}
                    for nm in ANAMES:
                        data_ap, idx_t = adata[nm]
                        dst = qp.tile([P, HBINS + 2], I16, tag=f"dst_{nm}")
                        sc_bi = nc.gpsimd.local_scatter(
                            dst[:, 0:HBINS], data_ap, idx_t[:],
                            channels=P, num_elems=HBINS, num_idxs=NIDX)
                        add_dep_helper(sc_bi.ins, lib_bi.ins, sync=True,
                                       reason="lib before scatter")
                        dsts[nm] = dst

                    # ---- per quarter: fill scans + interpolation ---------
                    for qh in range(2):
                        qb0 = h * HBINS + qh * QBINS
                        qs = slice(qh * QBINS, (qh + 1) * QBINS)
                        emt = qp.tile([P, QBINS], F32, tag="emt")
                        nc.vector.tensor_scalar(out=emt[:],
                                                in0=dsts["cp1"][:, qs],
                                                scalar1=0, scalar2=None,
                                                op0=A.is_equal)
                        filled = {}
                        for nm in ANAMES:
                            f = qp.tile([P, QBINS], F32, tag=f"fill_{nm}")
                            init = inits[nm] if (h == 0 and qh == 0) else carries[nm]
                            init_ap = init if isinstance(init, float) else init[:, 0:1]
                            nc.vector.tensor_tensor_scan(
                                f[:], emt[:], dsts[nm][:, qs], init_ap,
                                A.mult, A.add)
                            filled[nm] = f
                            cy = pers.tile([P, 1], F32, tag=f"carry_{nm}")
                            nc.vector.tensor_copy(out=cy[:],
                                                  in_=f[:, QBINS - 1:QBINS])
                            carries[nm] = cy

                        if debug:
                            for dnm, key in [("d_fcp1", "cp1"), ("d_fu", "u"),
                                             ("d_fdx", "dx"), ("d_fy1", "y1"),
                                             ("d_fy1n", "y1n")]:
                                nc.sync.dma_start(
                                    out=dbg[dnm][:, qb0:qb0 + QBINS],
                                    in_=filled[key][:])

                        # interpolation over grid m in [qb0, min(qb0+QBINS, M))
                        W = min(qb0 + QBINS, M) - qb0
                        if W <= 0:
                            continue
                        fsl = slice(0, W)
                        xcb = qp.tile([P, QBINS], F32, tag="xcb")
                        nc.sync.dma_start(
                            out=xcb[:, :W],
                            in_=xc_in[:, qb0:qb0 + W].to_broadcast([P, W]))
                        ma = qp.tile([P, QBINS], F32, tag="ma")
                        nc.vector.tensor_scalar(out=ma[:, :W], in0=xcb[:, :W],
                                                scalar1=x0[:, 0:1], scalar2=None,
                                                op0=A.is_ge)
                        scr1 = qp.tile([P, QBINS], F32, tag="scr1")
                        nc.vector.tensor_scalar(out=scr1[:, :W], in0=xcb[:, :W],
                                                scalar1=xlast[:, 0:1],
                                                scalar2=None, op0=A.is_le)
                        nc.vector.scalar_tensor_tensor(
                            out=ma[:, :W], in0=ma[:, :W],
                            scalar=float(1.0 / YSCALE), in1=scr1[:, :W],
                            op0=A.mult, op1=A.mult)
                        # x_lo = xmin + (cp1f - 2 + u)*dx ; us <- xc - x_lo
                        us = qp.tile([P, QBINS], F32, tag="us")
                        nc.scalar.activation(us[:, :W], filled["u"][:, fsl],
                                             AF.Copy, scale=float(dx / USCALE))
                        nc.vector.scalar_tensor_tensor(
                            out=us[:, :W], in0=filled["cp1"][:, fsl], scalar=dx,
                            in1=us[:, :W], op0=A.mult, op1=A.add)
                        nc.vector.scalar_tensor_tensor(
                            out=us[:, :W], in0=xcb[:, :W],
                            scalar=float(xmin - 2.0 * dx), in1=us[:, :W],
                            op0=A.subtract, op1=A.subtract)
                        # denom -> scr1b, recip -> scr2
                        scr1b = qp.tile([P, QBINS], F32, tag="scr1")
                        nc.vector.tensor_scalar(out=scr1b[:, :W],
                                                in0=filled["dx"][:, fsl],
                                                scalar1=0.0, scalar2=None,
                                                op0=A.is_equal)
                        nc.vector.scalar_tensor_tensor(
                            out=scr1b[:, :W], in0=filled["dx"][:, fsl],
                            scalar=float(1.0 / DXSCALE), in1=scr1b[:, :W],
                            op0=A.mult, op1=A.add)
                        nc.vector.tensor_scalar(out=scr1b[:, :W],
                                                in0=scr1b[:, :W],
                                                scalar1=1e-9, scalar2=None,
                                                op0=A.add)
                        scr2 = qp.tile([P, QBINS], F32, tag="scr2")
                        nc.vector.reciprocal(scr2[:, :W], scr1b[:, :W])
                        w_t = qp.tile([P, QBINS], F32, tag="w_t")
                        nc.vector.tensor_tensor(out=w_t[:, :W], in0=us[:, :W],
                                                in1=scr2[:, :W], op=A.mult)
                        nc.vector.tensor_scalar(out=w_t[:, :W], in0=w_t[:, :W],
                                                scalar1=1.0, scalar2=0.0,
                                                op0=A.min, op1=A.max)
                        for ynm, yc in [("y1", y1c), ("y2", y2c)]:
                            e = qp.tile([P, QBINS], F32, tag="scr2")
                            nc.vector.tensor_tensor(out=e[:, :W],
                                                    in0=filled[ynm + "n"][:, fsl],
                                                    in1=filled[ynm][:, fsl],
                                                    op=A.subtract)
                            nc.vector.tensor_tensor(out=e[:, :W], in0=w_t[:, :W],
                                                    in1=e[:, :W], op=A.mult)
                            nc.vector.tensor_tensor(out=e[:, :W], in0=e[:, :W],
                                                    in1=filled[ynm][:, fsl],
                                                    op=A.add)
                            nc.vector.tensor_tensor(out=yc[:, qb0:qb0 + W],
                                                    in0=e[:, :W], in1=ma[:, :W],
                                                    op=A.mult)
                            spt = sp.tile([P, 1], F32, tag="spt")
                            e2 = qp.tile([P, QBINS], F32, tag="scr2")
                            nc.scalar.activation(e2[:, :W], yc[:, qb0:qb0 + W],
                                                 AF.Square, accum_out=spt[:, 0:1])
                            nc.vector.tensor_tensor(out=sqacc[ynm][:],
                                                    in0=sqacc[ynm][:],
                                                    in1=spt[:], op=A.add)

            # ---- sq = mean(y^2) ------------------------------------------
            sqa = {}
            for ynm in ("y1", "y2"):
                s = pers.tile([P, 1], F32, tag=f"sqa_{ynm}")
                nc.vector.tensor_scalar(out=s[:], in0=sqacc[ynm][:],
                                        scalar1=float(1.0 / M), scalar2=None,
                                        op0=A.mult)
                sqa[ynm] = s

            if debug:
                nc.sync.dma_start(out=dbg["d_y1c"][:], in_=y1c[:, 0:M])
                nc.sync.dma_start(out=dbg["d_y2c"][:], in_=y2c[:, 0:M])
                nc.sync.dma_start(out=dbg["d_sq1"][:], in_=sqa["y1"][:])

            with (
                tc.tile_pool(name="ep", bufs=1) as ep,
                tc.tile_pool(name="rhsp", bufs=3) as rhsp,
            ):
                # ---- transposes to [m, rows] bf16 ------------------------
                ident = ep.tile([P, P], BF16, tag="ident")
                nc.sync.dma_start(out=ident[:], in_=id_in[:])
                y1T = ep.tile([P, MT], BF16, tag="y1T")
                y2T = ep.tile([P, MT], BF16, tag="y2T")
                for kt in range(KT):
                    for src, dstt in [(y1c, y1T), (y2c, y2T)]:
                        ps = pp.tile([P, P], BF16, tag="tps", space="PSUM")
                        nc.tensor.transpose(out=ps[:],
                                            in_=src[:, kt * P:(kt + 1) * P],
                                            identity=ident[:])
                        nc.vector.tensor_copy(out=dstt[:, kt * P:(kt + 1) * P],
                                              in_=ps[:])

                # ---- AllGather of y2T + sq2 hi/res (bf16) ----------------
                sq2hi = ep.tile([P, 1], BF16, tag="sq2hi")
                nc.vector.tensor_copy(out=sq2hi[:], in_=sqa["y2"][:])
                sq2hf = ep.tile([P, 1], F32, tag="sq2hf")
                nc.vector.tensor_copy(out=sq2hf[:], in_=sq2hi[:])
                sq2res = ep.tile([P, 1], BF16, tag="sq2res")
                nc.vector.tensor_tensor(out=sq2res[:], in0=sqa["y2"][:],
                                        in1=sq2hf[:], op=A.subtract)
                AGW = MT + 2  # 3074 per partition-row
                agin = dp.tile([P, AGW], BF16)
                agout = dp.tile([NCORES * P, AGW], BF16, addr_space="Shared")
                nc.sync.dma_start(out=agin[:, 0:MT], in_=y2T[:])
                nc.sync.dma_start(out=agin[:, MT:MT + 1], in_=sq2hi[:])
                nc.sync.dma_start(out=agin[:, MT + 1:AGW], in_=sq2res[:])
                nc.gpsimd.collective_compute(
                    "AllGather", A.bypass,
                    replica_groups=[list(range(NCORES))],
                    ins=[agin[:].opt()], outs=[agout[:].opt()])

                # ---- matmul: cross[i,j] = sum_m y1c[i,m]*y2c_all[j,m] ----
                cross = mmpp.tile([P, B], F32, space="PSUM")
                agv = agout[:].rearrange("(r p) f -> r p f", r=NCORES)
                for kt in range(KT):
                    rhs = rhsp.tile([P, B], BF16, tag="rhs")
                    nc.sync.dma_start(
                        out=rhs[:].rearrange("p (r f) -> p r f", r=NCORES),
                        in_=agv[:, :, kt * P:(kt + 1) * P]
                            .rearrange("r p f -> p r f"))
                    for jh in range(2):
                        nc.tensor.matmul(
                            cross[:, jh * 512:(jh + 1) * 512],
                            y1T[:, kt * P:(kt + 1) * P],
                            rhs[:, jh * 512:(jh + 1) * 512],
                            start=(kt == 0), stop=(kt == KT - 1))

                # ---- epilogue --------------------------------------------
                sq2hi_b = ep.tile([P, B], BF16, tag="sq2hi_b")
                nc.sync.dma_start(
                    out=sq2hi_b[:].rearrange("p (r f) -> p r f", r=NCORES),
                    in_=agv[:, :, MT:MT + 1].rearrange("r p f -> f r p")
                        .to_broadcast([P, NCORES, P]))
                sq2res_b = ep.tile([P, B], BF16, tag="sq2res_b")
                nc.sync.dma_start(
                    out=sq2res_b[:].rearrange("p (r f) -> p r f", r=NCORES),
                    in_=agv[:, :, MT + 1:MT + 2].rearrange("r p f -> f r p")
                        .to_broadcast([P, NCORES, P]))
                sq2g = ep.tile([P, B], F32, tag="sq2g")
                nc.vector.tensor_tensor(out=sq2g[:], in0=sq2hi_b[:],
                                        in1=sq2res_b[:], op=A.add)
                diff = ep.tile([P, B], F32, tag="diff")
                nc.vector.scalar_tensor_tensor(out=diff[:], in0=cross[:],
                                               scalar=float(-2.0 / M),
                                               in1=sq2g[:], op0=A.mult,
                                               op1=A.add)
                nc.vector.tensor_scalar(out=diff[:], in0=diff[:],
                                        scalar1=sqa["y1"][:, 0:1],
                                        scalar2=0.0, op0=A.add, op1=A.max)
                base = ep.tile([P, 1], F32, tag="base")
                nc.vector.tensor_tensor(out=base[:], in0=sqa["y1"][:],
                                        in1=sqa["y2"][:], op=A.add)
                nc.vector.tensor_scalar(out=base[:], in0=base[:], scalar1=1e-8,
                                        scalar2=None, op0=A.add)
                rbase = ep.tile([P, 1], F32, tag="rbase")
                nc.vector.reciprocal(rbase[:], base[:])
                nc.vector.scalar_tensor_tensor(out=diff[:], in0=diff[:],
                                               scalar=2.0,
                                               in1=rbase[:].to_broadcast([P, B]),
                                               op0=A.mult, op1=A.mult)
                lout = ep.tile([P, B], F32, tag="lout")
                nc.scalar.activation(lout[:], diff[:], AF.Sqrt)
                nc.sync.dma_start(out=o_out[:], in_=lout[:])

    nc.compile()
    return nc


def _host_prep(x):
    xmin = np.float32(x[:, 0].min())
    xmax = np.float32(x[:, -1].max())
    grid = np.linspace(np.float32(0.0), np.float32(1.0), M, dtype=np.float32)
    xc = (xmin + grid * (xmax - xmin)).astype(np.float32)[None, :]
    return xmin, xmax, xc


def kernel(x, y1, y2, debug=False, trace=False):
    import ml_dtypes
    from concourse.bass_utils import run_bass_kernel_spmd

    x = np.ascontiguousarray(x, dtype=np.float32)
    y1 = np.ascontiguousarray(y1, dtype=np.float32)
    y2 = np.ascontiguousarray(y2, dtype=np.float32)
    xmin, xmax, xc = _host_prep(x)
    ident = np.eye(P, dtype=ml_dtypes.bfloat16)

    nc = build_nc(float(xmin), float(xmax), debug=debug)
    in_maps = []
    for r in range(NCORES):
        rows = slice(r * R, (r + 1) * R)
        in_maps.append({"x": x[rows], "y1": y1[rows], "y2": y2[rows],
                        "xc": xc, "ident": ident})
    res = run_bass_kernel_spmd(nc, in_maps, core_ids=list(range(NCORES)),
                               trace=trace)
    out = np.concatenate([res.results[r]["out"] for r in range(NCORES)], axis=0)
    if debug or trace:
        return out, res
    return out



# revision 4
# speedup vs baseline: 1.7338x; 1.7338x over previous
# Bass/Trainium2 kernel for nn_L2PairwiceObjectiveFunction (pairwise L2 loss
# between per-row linear interpolations of two curve sets onto a common
# uniform grid).
#
# Full inputs: x, y1, y2 [1024, 8192] f32 (x sorted per row).
# Output: [1024, 1024] f32.
#
# Sharding: batch rows split across 8 NeuronCores (128 rows each, rows on
# SBUF partitions). The pairwise bilinear form uses an AllGather of the
# transposed [3072, 128] interpolated y2 grids (bf16) followed by a local
# PE matmul.
#
# Interpolation algorithm (searchsorted-free): the common grid is UNIFORM,
# so each data point's grid cell is computable elementwise:
# c[n] = floor((x[n]-xmin)/dx) + 1, clipped to [0, 3000]. For grid point m
# the bracketing segment is the last n with c[n] <= m. We scatter per-datum
# quantities (cell marker, frac(x), gap, y-lo, y-next; int16-quantized)
# into grid bins with gpsimd local_scatter (true per-partition indices;
# last-datum-per-bin enforced by a dedup mask so indices are unique), then
# fill empty bins with a carry-forward tensor_tensor_scan
# (state = empty*state + value). Interpolation is then pure elementwise
# work. Bin space is processed in two scatter halves x two scan/interp
# quarters to fit SBUF.

import numpy as np

B, N, M, NCORES = 1024, 8192, 3000, 8
R = B // NCORES  # 128 rows per core
P = 128
NBINS = 3004        # 2*HBINS bins (c clipped to [0, 3000])
HBINS = 1502        # bins per scatter half: [0,1502), [1502,3004)
QBINS = 751         # bins per scan/interp quarter
NIDX = 4608         # datum window per half (covers Binomial spread at ~11 sigma)
WOFF = (0, N - NIDX)   # window starts per half
WPAD = NIDX + 16    # padded quant-tile width (need NIDX+1 for shifted reads)
USCALE = 32766.0
YSCALE = 3000.0
DXSCALE = 1e7
DXCLIP = 3.2e-3
KT = 24             # matmul k-tiles; grid padded 3000 -> 3072
MT = KT * P
WB = 512            # stage-A column block


def build_nc(xmin, xmax, debug=False):
    import concourse.bacc as bacc
    import concourse.mybir as mybir
    from concourse.tile import TileContext
    from concourse import library_config
    from concourse.tile_rust import add_dep_helper

    F32, BF16, I16 = mybir.dt.float32, mybir.dt.bfloat16, mybir.dt.int16
    A = mybir.AluOpType
    AF = mybir.ActivationFunctionType

    dx = float((np.float32(xmax) - np.float32(xmin)) / np.float32(M - 1))
    inv_dx = float(np.float32(1.0) / np.float32(dx))

    nc = bacc.Bacc("TRN2", target_bir_lowering=False)
    x_in = nc.dram_tensor("x", [R, N], F32, kind="ExternalInput")
    y1_in = nc.dram_tensor("y1", [R, N], F32, kind="ExternalInput")
    y2_in = nc.dram_tensor("y2", [R, N], F32, kind="ExternalInput")
    xc_in = nc.dram_tensor("xc", [1, M], F32, kind="ExternalInput")
    id_in = nc.dram_tensor("ident", [P, P], BF16, kind="ExternalInput")
    o_out = nc.dram_tensor("out", [R, B], F32, kind="ExternalOutput")
    dbg = {}
    if debug:
        for nm, w, dt in [
            ("d_cp1", WPAD, I16), ("d_u16", WPAD, I16), ("d_dxq", WPAD, I16),
            ("d_y1q", WPAD, I16), ("d_idx", NIDX, I16),
            ("d_fcp1", NBINS, F32), ("d_fu", NBINS, F32), ("d_fdx", NBINS, F32),
            ("d_fy1", NBINS, F32), ("d_fy1n", NBINS, F32),
            ("d_y1c", M, BF16), ("d_y2c", M, BF16), ("d_sq1", 1, F32),
        ]:
            dbg[nm] = nc.dram_tensor(nm, [R, w], dt, kind="ExternalOutput")

    with TileContext(nc) as tc:
        with (
            tc.tile_pool(name="pers", bufs=1) as pers,
            tc.tile_pool(name="psum", bufs=2, space="PSUM") as pp,
            tc.tile_pool(name="mmpsum", bufs=1, space="PSUM") as mmpp,
            tc.tile_pool(name="dram", bufs=1, space="DRAM") as dp,
        ):
            lib_bi = nc.gpsimd.load_library(library_config.local_scatter)

            x0 = pers.tile([P, 1], F32, tag="x0")
            xlast = pers.tile([P, 1], F32, tag="xlast")
            nc.sync.dma_start(out=x0[:], in_=x_in[:, 0:1])
            nc.sync.dma_start(out=xlast[:], in_=x_in[:, N - 1:N])
            negone = pers.tile([P, 1], I16, tag="negone")
            nc.vector.memset(negone[:], -1)

            y1c = pers.tile([P, MT], BF16, tag="y1c")
            y2c = pers.tile([P, MT], BF16, tag="y2c")
            nc.vector.memset(y1c[:, M:], 0)
            nc.vector.memset(y2c[:, M:], 0)
            sqacc = {}
            for ynm in ("y1", "y2"):
                s = pers.tile([P, 1], F32, tag=f"sqacc_{ynm}")
                nc.vector.memset(s[:], 0)
                sqacc[ynm] = s
            carries = {}   # latest scan carry [P,1] per array
            inits = {}     # scan initials from first datum
            ANAMES = ("cp1", "u", "dx", "y1", "y1n", "y2", "y2n")

            with (
                tc.tile_pool(name="qp", bufs=1) as qp,
                tc.tile_pool(name="sp", bufs=2) as sp,
            ):
                for h in range(2):
                    woff = WOFF[h]
                    # ---- stage A: quantize datum window [woff, woff+NIDX] ----
                    cp1 = qp.tile([P, WPAD], I16, tag="cp1")
                    u16 = qp.tile([P, WPAD], I16, tag="u16")
                    dxq = qp.tile([P, WPAD], I16, tag="dxq")
                    y1q = qp.tile([P, WPAD], I16, tag="y1q")
                    y2q = qp.tile([P, WPAD], I16, tag="y2q")
                    for t in (cp1, u16, dxq, y1q, y2q):
                        nc.vector.memset(t[:, NIDX:], 0)
                    for bi in range(NIDX // WB):
                        lo = woff + bi * WB
                        wext = WB + 1 if lo + WB < N else WB
                        sl = slice(bi * WB, bi * WB + WB)
                        xb = sp.tile([P, WB + 1], F32, tag="xb")
                        nc.sync.dma_start(out=xb[:, :wext],
                                          in_=x_in[:, lo:lo + wext])
                        if wext == WB:
                            nc.vector.memset(xb[:, WB:], 0)
                        # t5 = (x - xmin)/dx + 0.5, clipped to [-0.5, 3000.0]
                        t5 = sp.tile([P, WB], F32, tag="t5")
                        nc.scalar.activation(t5[:], xb[:, :WB], AF.Copy,
                                             bias=float(0.5 - xmin * inv_dx),
                                             scale=inv_dx)
                        nc.vector.tensor_scalar(out=t5[:], in0=t5[:],
                                                scalar1=3000.0, scalar2=-0.5,
                                                op0=A.min, op1=A.max)
                        # c16 = round(t5) = floor(t)+1
                        c16b = sp.tile([P, WB], I16, tag="c16b")
                        nc.vector.tensor_copy(out=c16b[:], in_=t5[:])
                        nc.vector.tensor_scalar(out=cp1[:, sl], in0=c16b[:],
                                                scalar1=1, scalar2=None, op0=A.add)
                        # u16 = round((t5 + 0.5 - c16) * USCALE)
                        cf = sp.tile([P, WB], F32, tag="cf")
                        nc.scalar.copy(out=cf[:], in_=c16b[:])
                        nc.vector.scalar_tensor_tensor(out=t5[:], in0=t5[:],
                                                       scalar=0.5, in1=cf[:],
                                                       op0=A.add, op1=A.subtract)
                        nc.scalar.activation(u16[:, sl], t5[:], AF.Copy,
                                             scale=USCALE)
                        # gap -> dxq
                        xd = sp.tile([P, WB], F32, tag="xd")
                        nc.vector.tensor_tensor(out=xd[:], in0=xb[:, 1:WB + 1],
                                                in1=xb[:, :WB], op=A.subtract)
                        nc.vector.tensor_scalar(out=dxq[:, sl], in0=xd[:],
                                                scalar1=DXCLIP, scalar2=DXSCALE,
                                                op0=A.min, op1=A.mult)
                        # y quantization
                        yb = sp.tile([P, WB], F32, tag="yb")
                        nc.sync.dma_start(out=yb[:], in_=y1_in[:, lo:lo + WB])
                        nc.scalar.activation(y1q[:, sl], yb[:], AF.Copy,
                                             scale=YSCALE)
                        yb2 = sp.tile([P, WB], F32, tag="yb")
                        nc.sync.dma_start(out=yb2[:], in_=y2_in[:, lo:lo + WB])
                        nc.scalar.activation(y2q[:, sl], yb2[:], AF.Copy,
                                             scale=YSCALE)
                    if h == 0:
                        # col NIDX (shifted reads): quantize datum NIDX
                        xe = sp.tile([P, 4], F32, tag="xe")
                        nc.sync.dma_start(out=xe[:, 0:1], in_=x_in[:, NIDX:NIDX + 1])
                        nc.sync.dma_start(out=xe[:, 1:2], in_=y1_in[:, NIDX:NIDX + 1])
                        nc.sync.dma_start(out=xe[:, 2:3], in_=y2_in[:, NIDX:NIDX + 1])
                        t5e = sp.tile([P, 1], F32, tag="t5e")
                        nc.scalar.activation(t5e[:], xe[:, 0:1], AF.Copy,
                                             bias=float(0.5 - xmin * inv_dx),
                                             scale=inv_dx)
                        nc.vector.tensor_scalar(out=t5e[:], in0=t5e[:],
                                                scalar1=3000.0, scalar2=-0.5,
                                                op0=A.min, op1=A.max)
                        c16e = sp.tile([P, 1], I16, tag="c16e")
                        nc.vector.tensor_copy(out=c16e[:], in_=t5e[:])
                        nc.vector.tensor_scalar(out=cp1[:, NIDX:NIDX + 1],
                                                in0=c16e[:], scalar1=1,
                                                scalar2=None, op0=A.add)
                        nc.scalar.activation(y1q[:, NIDX:NIDX + 1], xe[:, 1:2],
                                             AF.Copy, scale=YSCALE)
                        nc.scalar.activation(y2q[:, NIDX:NIDX + 1], xe[:, 2:3],
                                             AF.Copy, scale=YSCALE)
                        # scan initials from datum 0
                        for nm, src in [("cp1", cp1[:, 0:1]), ("u", u16[:, 0:1]),
                                        ("y1", y1q[:, 0:1]), ("y1n", y1q[:, 1:2]),
                                        ("y2", y2q[:, 0:1]), ("y2n", y2q[:, 1:2])]:
                            it = pers.tile([P, 1], F32, tag=f"init_{nm}")
                            nc.vector.tensor_copy(out=it[:], in_=src)
                            inits[nm] = it
                        inits["dx"] = 0.0

                    # ---- dedup + bin-index mask --------------------------
                    neq = qp.tile([P, NIDX], I16, tag="neq")
                    nc.vector.tensor_tensor(out=neq[:], in0=cp1[:, 0:NIDX],
                                            in1=cp1[:, 1:NIDX + 1], op=A.not_equal)
                    if h == 1:
                        nc.vector.memset(neq[:, NIDX - 1:], 0)
                    idx = qp.tile([P, NIDX], I16, tag="idx")
                    nc.vector.memset(idx[:], 0)
                    nc.vector.copy_predicated(out=idx[:], mask=neq[:],
                                              data=cp1[:, 0:NIDX])
                    nc.vector.tensor_scalar(out=idx[:], in0=idx[:], scalar1=1,
                                            scalar2=None, op0=A.subtract)
                    sel = qp.tile([P, NIDX], I16, tag="neq")  # reuse slot
                    if h == 0:
                        nc.vector.tensor_scalar(out=sel[:], in0=idx[:],
                                                scalar1=HBINS - 1, scalar2=None,
                                                op0=A.is_gt)
                        nc.vector.copy_predicated(
                            out=idx[:], mask=sel[:],
                            data=negone[:].to_broadcast([P, NIDX]))
                    else:
                        nc.vector.tensor_scalar(out=sel[:], in0=idx[:],
                                                scalar1=HBINS - 1, scalar2=None,
                                                op0=A.is_le)
                        nc.vector.tensor_scalar(out=idx[:], in0=idx[:],
                                                scalar1=HBINS, scalar2=None,
                                                op0=A.subtract)
                        nc.vector.copy_predicated(
                            out=idx[:], mask=sel[:],
                            data=negone[:].to_broadcast([P, NIDX]))

                    if debug and h == 0:
                        for nm, t in [("d_cp1", cp1), ("d_u16", u16),
                                      ("d_dxq", dxq), ("d_y1q", y1q),
                                      ("d_idx", idx)]:
                            nc.sync.dma_start(out=dbg[nm][:], in_=t[:])

                    # ---- scatters (7 arrays into this half's bins) -------
                    # local_scatter mishandles data APs with a nonzero offset
                    # (drops some writes), so the "next-datum" arrays are
                    # scattered with a materialized shifted INDEX array
                    # instead: value y[j] goes to the bin of datum j-1.
                    idxp = qp.tile([P, NIDX], I16, tag="idxp")
                    nc.vector.memset(idxp[:, 0:1], -1)
                    nc.vector.tensor_copy(out=idxp[:, 1:NIDX],
                                          in_=idx[:, 0:NIDX - 1])
                    adata = {
                        "cp1": (cp1[:, 0:NIDX], idx), "u": (u16[:, 0:NIDX], idx),
                        "dx": (dxq[:, 0:NIDX], idx),
                        "y1": (y1q[:, 0:NIDX], idx),
                        "y1n": (y1q[:, 0:NIDX], idxp),
                        "y2": (y2q[:, 0:NIDX], idx),
                        "y2n": (y2q[:, 0:NIDX], idxp),
                    }
                    dsts = {}
                    for nm in ANAMES:
                        data_ap, idx_t = adata[nm]
                        dst = qp.tile([P, HBINS + 2], I16, tag=f"dst_{nm}")
                        sc_bi = nc.gpsimd.local_scatter(
                            dst[:, 0:HBINS], data_ap, idx_t[:],
                            channels=P, num_elems=HBINS, num_idxs=NIDX)
                        add_dep_helper(sc_bi.ins, lib_bi.ins, sync=True,
                                       reason="lib before scatter")
                        dsts[nm] = dst

                    # ---- per quarter: fill scans + interpolation ---------
                    for qh in range(2):
                        qb0 = h * HBINS + qh * QBINS
                        qs = slice(qh * QBINS, (qh + 1) * QBINS)
                        emt = qp.tile([P, QBINS], F32, tag="emt")
                        nc.vector.tensor_scalar(out=emt[:],
                                                in0=dsts["cp1"][:, qs],
                                                scalar1=0, scalar2=None,
                                                op0=A.is_equal)
                        filled = {}
                        for nm in ANAMES:
                            f = qp.tile([P, QBINS], F32, tag=f"fill_{nm}")
                            init = inits[nm] if (h == 0 and qh == 0) else carries[nm]
                            init_ap = init if isinstance(init, float) else init[:, 0:1]
                            nc.vector.tensor_tensor_scan(
                                f[:], emt[:], dsts[nm][:, qs], init_ap,
                                A.mult, A.add)
                            filled[nm] = f
                            cy = pers.tile([P, 1], F32, tag=f"carry_{nm}")
                            nc.vector.tensor_copy(out=cy[:],
                                                  in_=f[:, QBINS - 1:QBINS])
                            carries[nm] = cy

                        if debug:
                            for dnm, key in [("d_fcp1", "cp1"), ("d_fu", "u"),
                                             ("d_fdx", "dx"), ("d_fy1", "y1"),
                                             ("d_fy1n", "y1n")]:
                                nc.sync.dma_start(
                                    out=dbg[dnm][:, qb0:qb0 + QBINS],
                                    in_=filled[key][:])

                        # interpolation over grid m in [qb0, min(qb0+QBINS, M))
                        W = min(qb0 + QBINS, M) - qb0
                        if W <= 0:
                            continue
                        fsl = slice(0, W)
                        xcb = qp.tile([P, QBINS], F32, tag="xcb")
                        nc.sync.dma_start(
                            out=xcb[:, :W],
                            in_=xc_in[:, qb0:qb0 + W].to_broadcast([P, W]))
                        ma = qp.tile([P, QBINS], F32, tag="ma")
                        nc.vector.tensor_scalar(out=ma[:, :W], in0=xcb[:, :W],
                                                scalar1=x0[:, 0:1], scalar2=None,
                                                op0=A.is_ge)
                        scr1 = qp.tile([P, QBINS], F32, tag="scr1")
                        nc.vector.tensor_scalar(out=scr1[:, :W], in0=xcb[:, :W],
                                                scalar1=xlast[:, 0:1],
                                                scalar2=None, op0=A.is_le)
                        nc.vector.scalar_tensor_tensor(
                            out=ma[:, :W], in0=ma[:, :W],
                            scalar=float(1.0 / YSCALE), in1=scr1[:, :W],
                            op0=A.mult, op1=A.mult)
                        # x_lo = xmin + (cp1f - 2 + u)*dx ; us <- xc - x_lo
                        us = qp.tile([P, QBINS], F32, tag="us")
                        nc.scalar.activation(us[:, :W], filled["u"][:, fsl],
                                             AF.Copy, scale=float(dx / USCALE))
                        nc.vector.scalar_tensor_tensor(
                            out=us[:, :W], in0=filled["cp1"][:, fsl], scalar=dx,
                            in1=us[:, :W], op0=A.mult, op1=A.add)
                        nc.vector.scalar_tensor_tensor(
                            out=us[:, :W], in0=xcb[:, :W],
                            scalar=float(xmin - 2.0 * dx), in1=us[:, :W],
                            op0=A.subtract, op1=A.subtract)
                        # denom -> scr1b, recip -> scr2
                        scr1b = qp.tile([P, QBINS], F32, tag="scr1")
                        nc.vector.tensor_scalar(out=scr1b[:, :W],
                                                in0=filled["dx"][:, fsl],
                                                scalar1=0.0, scalar2=None,
                                                op0=A.is_equal)
                        nc.vector.scalar_tensor_tensor(
                            out=scr1b[:, :W], in0=filled["dx"][:, fsl],
                            scalar=float(1.0 / DXSCALE), in1=scr1b[:, :W],
                            op0=A.mult, op1=A.add)
                        nc.vector.tensor_scalar(out=scr1b[:, :W],
                                                in0=scr1b[:, :W],
                                                scalar1=1e-9, scalar2=None,
                                                op0=A.add)
                        scr2 = qp.tile([P, QBINS], F32, tag="scr2")
                        nc.vector.reciprocal(scr2[:, :W], scr1b[:, :W])
                        w_t = qp.tile([P, QBINS], F32, tag="w_t")
                        nc.vector.tensor_tensor(out=w_t[:, :W], in0=us[:, :W],
                                                in1=scr2[:, :W], op=A.mult)
                        nc.vector.tensor_scalar(out=w_t[:, :W], in0=w_t[:, :W],
                                                scalar1=1.0, scalar2=0.0,
                                                op0=A.min, op1=A.max)
                        for ynm, yc in [("y1", y1c), ("y2", y2c)]:
                            e = qp.tile([P, QBINS], F32, tag="scr2")
                            nc.vector.tensor_tensor(out=e[:, :W],
                                                    in0=filled[ynm + "n"][:, fsl],
                                                    in1=filled[ynm][:, fsl],
                                                    op=A.subtract)
                            nc.vector.tensor_tensor(out=e[:, :W], in0=w_t[:, :W],
                                                    in1=e[:, :W], op=A.mult)
                            nc.vector.tensor_tensor(out=e[:, :W], in0=e[:, :W],
                                                    in1=filled[ynm][:, fsl],
                                                    op=A.add)
                            nc.vector.tensor_tensor(out=yc[:, qb0:qb0 + W],
                                                    in0=e[:, :W], in1=ma[:, :W],
                                                    op=A.mult)
                            spt = sp.tile([P, 1], F32, tag="spt")
                            e2 = qp.tile([P, QBINS], F32, tag="scr2")
                            nc.scalar.activation(e2[:, :W], yc[:, qb0:qb0 + W],
                                                 AF.Square, accum_out=spt[:, 0:1])
                            nc.vector.tensor_tensor(out=sqacc[ynm][:],
                                                    in0=sqacc[ynm][:],
                                                    in1=spt[:], op=A.add)

            # ---- sq = mean(y^2) ------------------------------------------
            sqa = {}
            for ynm in ("y1", "y2"):
                s = pers.tile([P, 1], F32, tag=f"sqa_{ynm}")
                nc.vector.tensor_scalar(out=s[:], in0=sqacc[ynm][:],
                                        scalar1=float(1.0 / M), scalar2=None,
                                        op0=A.mult)
                sqa[ynm] = s

            if debug:
                nc.sync.dma_start(out=dbg["d_y1c"][:], in_=y1c[:, 0:M])
                nc.sync.dma_start(out=dbg["d_y2c"][:], in_=y2c[:, 0:M])
                nc.sync.dma_start(out=dbg["d_sq1"][:], in_=sqa["y1"][:])

            with (
                tc.tile_pool(name="ep", bufs=1) as ep,
                tc.tile_pool(name="rhsp", bufs=3) as rhsp,
            ):
                # ---- transposes to [m, rows] bf16 ------------------------
                ident = ep.tile([P, P], BF16, tag="ident")
                nc.sync.dma_start(out=ident[:], in_=id_in[:])
                y1T = ep.tile([P, MT], BF16, tag="y1T")
                y2T = ep.tile([P, MT], BF16, tag="y2T")
                for kt in range(KT):
                    for src, dstt in [(y1c, y1T), (y2c, y2T)]:
                        ps = pp.tile([P, P], BF16, tag="tps", space="PSUM")
                        nc.tensor.transpose(out=ps[:],
                                            in_=src[:, kt * P:(kt + 1) * P],
                                            identity=ident[:])
                        nc.vector.tensor_copy(out=dstt[:, kt * P:(kt + 1) * P],
                                              in_=ps[:])

                # ---- fold -1500*sq2[j] into spare matmul k-slots ---------
                # Slots m=3008,3009 (partition 64 of the last k-tile;
                # vector ops need partition base 0/32/64/96) carry
                # bf16 hi/res halves of v = -(M/2)*sq2; with lhs slots = 1,
                # cross picks up -1500*sq2[j], and (-2/M)*cross then
                # includes +sq2[j] exactly -- no [B]-wide broadcast needed.
                sq2q = ep.tile([P, 2], BF16, tag="sq2q")
                nc.vector.tensor_scalar(out=sq2q[:, 0:1], in0=sqa["y2"][:],
                                        scalar1=float(-M / 2.0), scalar2=None,
                                        op0=A.mult)
                qhf = ep.tile([P, 1], F32, tag="qhf")
                nc.scalar.copy(out=qhf[:], in_=sq2q[:, 0:1])
                nc.vector.scalar_tensor_tensor(out=sq2q[:, 1:2],
                                               in0=sqa["y2"][:],
                                               scalar=float(-M / 2.0),
                                               in1=qhf[:], op0=A.mult,
                                               op1=A.subtract)
                scrd = dp.tile([P, 2], BF16)
                nc.sync.dma_start(out=scrd[:], in_=sq2q[:])
                nc.sync.dma_start(out=y2T[64:66, (KT - 1) * P:KT * P],
                                  in_=scrd[:].rearrange("i q -> q i"))
                nc.vector.memset(y1T[64:66, (KT - 1) * P:KT * P], 1.0)

                # ---- AllGather of y2T (bf16) -----------------------------
                agin = dp.tile([P, MT], BF16)
                agout = dp.tile([NCORES * P, MT], BF16, addr_space="Shared")
                nc.sync.dma_start(out=agin[:], in_=y2T[:])
                nc.gpsimd.collective_compute(
                    "AllGather", A.bypass,
                    replica_groups=[list(range(NCORES))],
                    ins=[agin[:].opt()], outs=[agout[:].opt()])

                # ---- matmul: cross[i,j] = sum_m y1c[i,m]*y2c_all[j,m] ----
                cross = mmpp.tile([P, B], F32, space="PSUM")
                agv = agout[:].rearrange("(r p) f -> r p f", r=NCORES)
                for kt in range(KT):
                    rhs = rhsp.tile([P, B], BF16, tag="rhs")
                    nc.sync.dma_start(
                        out=rhs[:].rearrange("p (r f) -> p r f", r=NCORES),
                        in_=agv[:, :, kt * P:(kt + 1) * P]
                            .rearrange("r p f -> p r f"))
                    for jh in range(2):
                        nc.tensor.matmul(
                            cross[:, jh * 512:(jh + 1) * 512],
                            y1T[:, kt * P:(kt + 1) * P],
                            rhs[:, jh * 512:(jh + 1) * 512],
                            start=(kt == 0), stop=(kt == KT - 1))

                # ---- epilogue --------------------------------------------
                diff = ep.tile([P, B], F32, tag="diff")
                nc.vector.tensor_scalar(out=diff[:], in0=cross[:],
                                        scalar1=float(-2.0 / M),
                                        scalar2=None, op0=A.mult)
                nc.vector.tensor_scalar(out=diff[:], in0=diff[:],
                                        scalar1=sqa["y1"][:, 0:1],
                                        scalar2=0.0, op0=A.add, op1=A.max)
                base = ep.tile([P, 1], F32, tag="base")
                nc.vector.tensor_tensor(out=base[:], in0=sqa["y1"][:],
                                        in1=sqa["y2"][:], op=A.add)
                nc.vector.tensor_scalar(out=base[:], in0=base[:], scalar1=1e-8,
                                        scalar2=None, op0=A.add)
                rbase = ep.tile([P, 1], F32, tag="rbase")
                nc.vector.reciprocal(rbase[:], base[:])
                nc.vector.scalar_tensor_tensor(out=diff[:], in0=diff[:],
                                               scalar=2.0,
                                               in1=rbase[:].to_broadcast([P, B]),
                                               op0=A.mult, op1=A.mult)
                lout = ep.tile([P, B], F32, tag="lout")
                nc.scalar.activation(lout[:], diff[:], AF.Sqrt)
                nc.sync.dma_start(out=o_out[:], in_=lout[:])

    nc.compile()
    return nc


def _host_prep(x):
    xmin = np.float32(x[:, 0].min())
    xmax = np.float32(x[:, -1].max())
    grid = np.linspace(np.float32(0.0), np.float32(1.0), M, dtype=np.float32)
    xc = (xmin + grid * (xmax - xmin)).astype(np.float32)[None, :]
    return xmin, xmax, xc


def kernel(x, y1, y2, debug=False, trace=False):
    import ml_dtypes
    from concourse.bass_utils import run_bass_kernel_spmd

    x = np.ascontiguousarray(x, dtype=np.float32)
    y1 = np.ascontiguousarray(y1, dtype=np.float32)
    y2 = np.ascontiguousarray(y2, dtype=np.float32)
    xmin, xmax, xc = _host_prep(x)
    ident = np.eye(P, dtype=ml_dtypes.bfloat16)

    nc = build_nc(float(xmin), float(xmax), debug=debug)
    in_maps = []
    for r in range(NCORES):
        rows = slice(r * R, (r + 1) * R)
        in_maps.append({"x": x[rows], "y1": y1[rows], "y2": y2[rows],
                        "xc": xc, "ident": ident})
    res = run_bass_kernel_spmd(nc, in_maps, core_ids=list(range(NCORES)),
                               trace=trace)
    out = np.concatenate([res.results[r]["out"] for r in range(NCORES)], axis=0)
    if debug or trace:
        return out, res
    return out



# revision 7
# speedup vs baseline: 1.9025x; 1.0973x over previous
# Bass/Trainium2 kernel for nn_L2PairwiceObjectiveFunction (pairwise L2 loss
# between per-row linear interpolations of two curve sets onto a common
# uniform grid).
#
# Full inputs: x, y1, y2 [1024, 8192] f32 (x sorted per row).
# Output: [1024, 1024] f32.
#
# Sharding: batch rows split across 8 NeuronCores (128 rows each, rows on
# SBUF partitions). The pairwise bilinear form uses an AllGather of the
# transposed [3072, 128] interpolated y2 grids (bf16) followed by a local
# PE matmul.
#
# Interpolation: the common grid is UNIFORM, so each data point's grid cell
# is computable elementwise: c[j] = floor((x[j]-xmin)/dx) + 1. For grid
# point m the bracketing segment is the last j with c[j] <= m. Per segment
# the interpolant is linear in g: y(g_m) = A' + Bd*(m - c_j) where
# A' = y_j + B*(g_{c_j} - x_j) (value at the segment's own grid point),
# Bd = B*dx, B = dy/(gap+1e-9). We scatter int16-quantized (A', Bd) for y1
# and y2 (4 arrays) into grid bins with gpsimd local_scatter (per-partition
# indices; last-datum-per-bin dedup keeps indices unique), fill empty bins
# with a carry-forward tensor_tensor_scan, recover (m - c_j) as the carry
# "age" via a second scan form, then interpolate elementwise. Bin space is
# processed in two scatter halves x two scan/interp quarters to fit SBUF.

import numpy as np

B, N, M, NCORES = 1024, 8192, 3000, 8
R = B // NCORES  # 128 rows per core
P = 128
NBINS = 3004        # bins (c in [0, 3001])
HBINS = 1502        # bins per scatter half: [0,1502), [1502,3004)
QBINS = 751         # bins per scan/interp quarter
NIDX = 4608         # datum window per half (covers Binomial spread at ~11 sigma)
WOFF = (0, N - NIDX)   # window starts per half
CPAD = N + 16       # padded cell-array width (need N+1 for shifted reads)
SA = 1489.0         # A' quantization scale (|A'| <= ~5.6 -> 8.3k of 16383)
AOFF = 16384.0      # A' offset so filled bins are nonzero (empty marker = 0)
SB = 2978.0         # Bd quantization scale (|Bd| clamped to 32725/SB ~ 10.99)
BCLIP = 32725.0
E6 = 1e6            # gap prescale so Reciprocal input is in [1e-3, ~2e3]
KT = 24             # matmul k-tiles; grid padded 3000 -> 3072
MT = KT * P
WB = 1024           # stage-A column block
NB = N // WB


def build_nc(xmin, xmax, debug=False):
    import concourse.bacc as bacc
    import concourse.mybir as mybir
    from concourse.tile import TileContext
    from concourse import library_config
    from concourse.tile_rust import add_dep_helper

    F32, BF16, I16 = mybir.dt.float32, mybir.dt.bfloat16, mybir.dt.int16
    A = mybir.AluOpType
    AF = mybir.ActivationFunctionType

    dx = float((np.float32(xmax) - np.float32(xmin)) / np.float32(M - 1))
    inv_dx = float(np.float32(1.0) / np.float32(dx))

    nc = bacc.Bacc("TRN2", target_bir_lowering=False)
    x_in = nc.dram_tensor("x", [R, N], F32, kind="ExternalInput")
    y1_in = nc.dram_tensor("y1", [R, N], F32, kind="ExternalInput")
    y2_in = nc.dram_tensor("y2", [R, N], F32, kind="ExternalInput")
    xc_in = nc.dram_tensor("xc", [1, M], F32, kind="ExternalInput")
    id_in = nc.dram_tensor("ident", [P, P], BF16, kind="ExternalInput")
    o_out = nc.dram_tensor("out", [R, B], F32, kind="ExternalOutput")
    dbg = {}
    if debug:
        for nm, w, dt in [
            ("d_cp1", CPAD, I16), ("d_a1q", N, I16), ("d_b1q", N, I16),
            ("d_fa1", NBINS, F32), ("d_fb1", NBINS, F32),
            ("d_age", NBINS, F32),
            ("d_y1c", M, BF16), ("d_y2c", M, BF16), ("d_sq1", 1, F32),
        ]:
            dbg[nm] = nc.dram_tensor(nm, [R, w], dt, kind="ExternalOutput")

    with TileContext(nc) as tc:
        with (
            tc.tile_pool(name="pers", bufs=1) as pers,
            tc.tile_pool(name="psum", bufs=2, space="PSUM") as pp,
            tc.tile_pool(name="mmpsum", bufs=1, space="PSUM") as mmpp,
            tc.tile_pool(name="dram", bufs=1, space="DRAM") as dp,
        ):
            lib_bi = nc.gpsimd.load_library(library_config.local_scatter)

            x0 = pers.tile([P, 1], F32, tag="x0")
            xlast = pers.tile([P, 1], F32, tag="xlast")
            nc.sync.dma_start(out=x0[:], in_=x_in[:, 0:1])
            nc.sync.dma_start(out=xlast[:], in_=x_in[:, N - 1:N])

            y1c = pers.tile([P, MT], BF16, tag="y1c")
            y2c = pers.tile([P, MT], BF16, tag="y2c")
            nc.vector.memset(y1c[:, M:], 0)
            nc.vector.memset(y2c[:, M:], 0)
            sqacc = {}
            for ynm in ("y1", "y2"):
                s = pers.tile([P, 1], F32, tag=f"sqacc_{ynm}")
                nc.vector.memset(s[:], 0)
                sqacc[ynm] = s
            carries = {}   # latest scan carry [P,1] per array
            inits = {}     # scan initials from first datum
            ANAMES = ("age", "a1", "b1", "a2", "b2")

            with (
                tc.tile_pool(name="qp", bufs=1) as qp,
                tc.tile_pool(name="ldp", bufs=2) as ldp,
                tc.tile_pool(name="sp", bufs=1) as sp,
            ):
                # ---- stage A: per-datum cells + interp coefficients ------
                cp1 = qp.tile([P, CPAD], I16, tag="cp1")   # c[j] + 1
                a1q = qp.tile([P, N], I16, tag="a1q")
                b1q = qp.tile([P, N], I16, tag="b1q")
                a2q = qp.tile([P, N], I16, tag="a2q")
                b2q = qp.tile([P, N], I16, tag="b2q")
                nc.vector.memset(cp1[:, N:], 30000)
                QARR = {"a1": a1q, "b1": b1q, "a2": a2q, "b2": b2q}

                for bi in range(NB):
                    lo = bi * WB
                    wext = WB + 1 if lo + WB < N else WB
                    sl = slice(lo, lo + WB)
                    xb = ldp.tile([P, WB + 1], F32, tag="xb")
                    yb1 = ldp.tile([P, WB + 1], F32, tag="yb1")
                    yb2 = ldp.tile([P, WB + 1], F32, tag="yb2")
                    nc.sync.dma_start(out=xb[:, :wext], in_=x_in[:, lo:lo + wext])
                    nc.sync.dma_start(out=yb1[:, :wext], in_=y1_in[:, lo:lo + wext])
                    nc.sync.dma_start(out=yb2[:, :wext], in_=y2_in[:, lo:lo + wext])
                    if wext == WB:
                        nc.vector.memset(xb[:, WB:], 0)
                        nc.vector.memset(yb1[:, WB:], 0)
                        nc.vector.memset(yb2[:, WB:], 0)
                    # t5 = (x - xmin)/dx + 1.5, clipped to [0.5, 3001]
                    t5 = sp.tile([P, WB], F32, tag="t5")
                    nc.scalar.activation(t5[:], xb[:, :WB], AF.Copy,
                                         bias=float(1.5 - xmin * inv_dx),
                                         scale=inv_dx)
                    nc.vector.tensor_scalar(out=t5[:], in0=t5[:],
                                            scalar1=3001.0, scalar2=0.5,
                                            op0=A.min, op1=A.max)
                    # cp1 = round(t5) = c + 1
                    nc.vector.tensor_copy(out=cp1[:, sl], in_=t5[:])
                    cf = sp.tile([P, WB], F32, tag="cf")
                    nc.scalar.copy(out=cf[:], in_=cp1[:, sl])
                    gd = sp.tile([P, WB], F32, tag="gd")
                    nc.vector.tensor_tensor(out=gd[:], in0=cf[:], in1=t5[:],
                                            op=A.subtract)
                    # gx = (g_c - x) * 1e6 = (gd + 0.5) * dx * 1e6
                    gx = sp.tile([P, WB], F32, tag="t5")  # t5 dead
                    nc.scalar.activation(gx[:], gd[:], AF.Copy,
                                         bias=float(0.5 * dx * E6),
                                         scale=float(dx * E6))
                    gap = sp.tile([P, WB], F32, tag="gap")
                    nc.vector.tensor_tensor(out=gap[:], in0=xb[:, 1:WB + 1],
                                            in1=xb[:, :WB], op=A.subtract)
                    # r6 = 1/(gap*1e6 + 1e-3) = recip(gap + 1e-9) * 1e-6
                    gape = sp.tile([P, WB], F32, tag="cf")  # cf dead
                    nc.scalar.activation(gape[:], gap[:], AF.Copy,
                                         bias=1e-3, scale=float(E6))
                    r6 = sp.tile([P, WB], F32, tag="r6")
                    nc.vector.reciprocal_approx_fast(out=r6[:], in_=gape[:])
                    for ynm, yb, aq, bq in (("1", yb1, a1q, b1q),
                                            ("2", yb2, a2q, b2q)):
                        dy = sp.tile([P, WB], F32, tag="gap")  # gap dead
                        nc.vector.tensor_tensor(out=dy[:], in0=yb[:, 1:WB + 1],
                                                in1=yb[:, :WB], op=A.subtract)
                        bb = sp.tile([P, WB], F32, tag="gd")  # gd dead
                        nc.vector.tensor_tensor(out=bb[:], in0=dy[:], in1=r6[:],
                                                op=A.mult)
                        # A' = y + B*gx
                        t6 = sp.tile([P, WB], F32, tag="t6")
                        nc.vector.tensor_tensor(out=t6[:], in0=bb[:], in1=gx[:],
                                                op=A.mult)
                        nc.vector.tensor_tensor(out=t6[:], in0=t6[:],
                                                in1=yb[:, :WB], op=A.add)
                        nc.scalar.activation(aq[:, sl], t6[:], AF.Copy,
                                             bias=AOFF, scale=SA)
                        # Bd*SB = B*dx*SB, clamped to +-BCLIP
                        bt = sp.tile([P, WB], F32, tag="gap")  # dy dead
                        nc.vector.tensor_scalar(out=bt[:], in0=bb[:],
                                                scalar1=float(E6 * dx * SB),
                                                scalar2=BCLIP,
                                                op0=A.mult, op1=A.min)
                        nc.vector.tensor_scalar(out=bq[:, sl], in0=bt[:],
                                                scalar1=-BCLIP, scalar2=None,
                                                op0=A.max)

                # scan initials from datum 0 (flat extension: B = 0)
                for nm, src in [("a1", a1q[:, 0:1]), ("a2", a2q[:, 0:1])]:
                    it = pers.tile([P, 1], F32, tag=f"init_{nm}")
                    nc.vector.tensor_copy(out=it[:], in_=src)
                    inits[nm] = it
                inits["b1"] = 0.0
                inits["b2"] = 0.0
                inits["age"] = 0.0

                if debug:
                    nc.sync.dma_start(out=dbg["d_cp1"][:], in_=cp1[:])
                    nc.sync.dma_start(out=dbg["d_a1q"][:], in_=a1q[:])
                    nc.sync.dma_start(out=dbg["d_b1q"][:], in_=b1q[:])

                for h in range(2):
                    woff = WOFF[h]
                    # ---- dedup + scatter index for this half -------------
                    neq = qp.tile([P, NIDX], I16, tag="neq")
                    nc.vector.tensor_tensor(out=neq[:],
                                            in0=cp1[:, woff:woff + NIDX],
                                            in1=cp1[:, woff + 1:woff + NIDX + 1],
                                            op=A.not_equal)
                    if h == 1:
                        # second-to-last datum always survives; last never
                        nc.vector.memset(neq[:, NIDX - 2:NIDX - 1], 1)
                        nc.vector.memset(neq[:, NIDX - 1:], 0)
                    ix = qp.tile([P, NIDX], I16, tag="idx")
                    nc.vector.memset(ix[:], 0)
                    nc.vector.copy_predicated(out=ix[:], mask=neq[:],
                                              data=cp1[:, woff:woff + NIDX])
                    if h == 0:
                        # bin = c = cp1 - 1; bins > 1501 pushed negative
                        nc.vector.tensor_scalar(out=ix[:], in0=ix[:], scalar1=1,
                                                scalar2=None, op0=A.subtract)
                        sel = qp.tile([P, NIDX], I16, tag="neq")  # reuse slot
                        nc.vector.tensor_scalar(out=sel[:], in0=ix[:],
                                                scalar1=HBINS - 1, scalar2=None,
                                                op0=A.is_gt)
                        nc.vector.scalar_tensor_tensor(out=ix[:], in0=sel[:],
                                                       scalar=-32000.0,
                                                       in1=ix[:], op0=A.mult,
                                                       op1=A.add)
                    else:
                        # bin = c - 1502; bins < 1502 go negative (ignored)
                        nc.vector.tensor_scalar(out=ix[:], in0=ix[:],
                                                scalar1=HBINS + 1, scalar2=None,
                                                op0=A.subtract)

                    # ---- 4 scatters --------------------------------------
                    dsts = {}
                    for nm in ("a1", "b1", "a2", "b2"):
                        dst = qp.tile([P, HBINS + 2], I16, tag=f"dst_{nm}")
                        sc_bi = nc.gpsimd.local_scatter(
                            dst[:, 0:HBINS],
                            QARR[nm][:, woff:woff + NIDX], ix[:],
                            channels=P, num_elems=HBINS, num_idxs=NIDX)
                        add_dep_helper(sc_bi.ins, lib_bi.ins, sync=True,
                                       reason="lib before scatter")
                        dsts[nm] = dst

                    # ---- per quarter: fill scans + interpolation ---------
                    for qh in range(2):
                        qb0 = h * HBINS + qh * QBINS
                        qs = slice(qh * QBINS, (qh + 1) * QBINS)
                        first = (h == 0 and qh == 0)
                        emt = qp.tile([P, QBINS], F32, tag="emt")
                        nc.vector.tensor_scalar(out=emt[:],
                                                in0=dsts["a1"][:, qs],
                                                scalar1=0, scalar2=None,
                                                op0=A.is_equal)
                        filled = {}
                        for nm in ANAMES:
                            f = qp.tile([P, QBINS], F32, tag=f"fill_{nm}")
                            init = inits[nm] if first else carries[nm]
                            init_ap = (init if isinstance(init, float)
                                       else init[:, 0:1])
                            src = emt[:] if nm == "age" else dsts[nm][:, qs]
                            nc.vector.tensor_tensor_scan(
                                f[:], emt[:], src, init_ap, A.mult, A.add)
                            filled[nm] = f
                            cy = pers.tile([P, 1], F32, tag=f"carry_{nm}")
                            nc.vector.tensor_copy(out=cy[:],
                                                  in_=f[:, QBINS - 1:QBINS])
                            carries[nm] = cy

                        if debug:
                            for dnm, key in [("d_fa1", "a1"), ("d_fb1", "b1"),
                                             ("d_age", "age")]:
                                nc.sync.dma_start(
                                    out=dbg[dnm][:, qb0:qb0 + QBINS],
                                    in_=filled[key][:])

                        # interpolation over grid m in [qb0, min(qb0+QBINS, M))
                        W = min(qb0 + QBINS, M) - qb0
                        if W <= 0:
                            continue
                        fsl = slice(0, W)
                        xcb = qp.tile([P, QBINS], F32, tag="xcb")
                        nc.sync.dma_start(
                            out=xcb[:, :W],
                            in_=xc_in[:, qb0:qb0 + W].to_broadcast([P, W]))
                        ma = qp.tile([P, QBINS], F32, tag="ma")
                        nc.vector.tensor_scalar(out=ma[:, :W], in0=xcb[:, :W],
                                                scalar1=x0[:, 0:1], scalar2=None,
                                                op0=A.is_ge)
                        scr1 = qp.tile([P, QBINS], F32, tag="scr1")
                        nc.vector.tensor_scalar(out=scr1[:, :W], in0=xcb[:, :W],
                                                scalar1=xlast[:, 0:1],
                                                scalar2=None, op0=A.is_le)
                        nc.vector.tensor_tensor(out=ma[:, :W], in0=ma[:, :W],
                                                in1=scr1[:, :W], op=A.mult)
                        for ynm, yc in (("1", y1c), ("2", y2c)):
                            # y = (fa - AOFF)/SA + (fb/SB)*age
                            t1 = qp.tile([P, QBINS], F32, tag="scr1")
                            nc.vector.scalar_tensor_tensor(
                                out=t1[:, :W], in0=filled["b" + ynm][:, fsl],
                                scalar=float(1.0 / SB),
                                in1=filled["age"][:, fsl],
                                op0=A.mult, op1=A.mult)
                            t2 = qp.tile([P, QBINS], F32, tag="emt")
                            nc.vector.tensor_scalar(
                                out=t2[:, :W], in0=filled["a" + ynm][:, fsl],
                                scalar1=-AOFF, scalar2=float(1.0 / SA),
                                op0=A.add, op1=A.mult)
                            nc.vector.tensor_tensor(out=t2[:, :W],
                                                    in0=t2[:, :W],
                                                    in1=t1[:, :W], op=A.add)
                            nc.vector.tensor_tensor(out=yc[:, qb0:qb0 + W],
                                                    in0=t2[:, :W],
                                                    in1=ma[:, :W], op=A.mult)
                            spt = qp.tile([P, 1], F32, tag="spt")
                            e2 = qp.tile([P, QBINS], F32, tag="emt")
                            nc.scalar.activation(e2[:, :W], yc[:, qb0:qb0 + W],
                                                 AF.Square,
                                                 accum_out=spt[:, 0:1])
                            nc.vector.tensor_tensor(out=sqacc["y" + ynm][:],
                                                    in0=sqacc["y" + ynm][:],
                                                    in1=spt[:], op=A.add)

            # ---- sq = mean(y^2) ------------------------------------------
            sqa = {}
            for ynm in ("y1", "y2"):
                s = pers.tile([P, 1], F32, tag=f"sqa_{ynm}")
                nc.vector.tensor_scalar(out=s[:], in0=sqacc[ynm][:],
                                        scalar1=float(1.0 / M), scalar2=None,
                                        op0=A.mult)
                sqa[ynm] = s

            if debug:
                nc.sync.dma_start(out=dbg["d_y1c"][:], in_=y1c[:, 0:M])
                nc.sync.dma_start(out=dbg["d_y2c"][:], in_=y2c[:, 0:M])
                nc.sync.dma_start(out=dbg["d_sq1"][:], in_=sqa["y1"][:])

            with (
                tc.tile_pool(name="ep", bufs=1) as ep,
                tc.tile_pool(name="rhsp", bufs=3) as rhsp,
            ):
                # ---- transposes to [m, rows] bf16 ------------------------
                ident = ep.tile([P, P], BF16, tag="ident")
                nc.sync.dma_start(out=ident[:], in_=id_in[:])
                y1T = ep.tile([P, MT], BF16, tag="y1T")
                y2T = ep.tile([P, MT], BF16, tag="y2T")
                for kt in range(KT):
                    for src, dstt in [(y1c, y1T), (y2c, y2T)]:
                        ps = pp.tile([P, P], BF16, tag="tps", space="PSUM")
                        nc.tensor.transpose(out=ps[:],
                                            in_=src[:, kt * P:(kt + 1) * P],
                                            identity=ident[:])
                        nc.vector.tensor_copy(out=dstt[:, kt * P:(kt + 1) * P],
                                              in_=ps[:])

                # ---- fold -1500*sq2[j] into spare matmul k-slots ---------
                # Slots m=3008,3009 (partition 64 of the last k-tile;
                # vector ops need partition base 0/32/64/96) carry
                # bf16 hi/res halves of v = -(M/2)*sq2; with lhs slots = 1,
                # cross picks up -1500*sq2[j], and (-2/M)*cross then
                # includes +sq2[j] exactly -- no [B]-wide broadcast needed.
                sq2q = ep.tile([P, 2], BF16, tag="sq2q")
                nc.vector.tensor_scalar(out=sq2q[:, 0:1], in0=sqa["y2"][:],
                                        scalar1=float(-M / 2.0), scalar2=None,
                                        op0=A.mult)
                qhf = ep.tile([P, 1], F32, tag="qhf")
                nc.scalar.copy(out=qhf[:], in_=sq2q[:, 0:1])
                nc.vector.scalar_tensor_tensor(out=sq2q[:, 1:2],
                                               in0=sqa["y2"][:],
                                               scalar=float(-M / 2.0),
                                               in1=qhf[:], op0=A.mult,
                                               op1=A.subtract)
                scrd = dp.tile([P, 2], BF16)
                nc.sync.dma_start(out=scrd[:], in_=sq2q[:])
                nc.sync.dma_start(out=y2T[64:66, (KT - 1) * P:KT * P],
                                  in_=scrd[:].rearrange("i q -> q i"))
                nc.vector.memset(y1T[64:66, (KT - 1) * P:KT * P], 1.0)

                # ---- AllGather of y2T (bf16) -----------------------------
                agin = dp.tile([P, MT], BF16)
                agout = dp.tile([NCORES * P, MT], BF16, addr_space="Shared")
                nc.sync.dma_start(out=agin[:], in_=y2T[:])
                nc.gpsimd.collective_compute(
                    "AllGather", A.bypass,
                    replica_groups=[list(range(NCORES))],
                    ins=[agin[:].opt()], outs=[agout[:].opt()])

                # ---- matmul: cross[i,j] = sum_m y1c[i,m]*y2c_all[j,m] ----
                cross = mmpp.tile([P, B], F32, space="PSUM")
                agv = agout[:].rearrange("(r p) f -> r p f", r=NCORES)
                for kt in range(KT):
                    rhs = rhsp.tile([P, B], BF16, tag="rhs")
                    nc.sync.dma_start(
                        out=rhs[:].rearrange("p (r f) -> p r f", r=NCORES),
                        in_=agv[:, :, kt * P:(kt + 1) * P]
                            .rearrange("r p f -> p r f"))
                    for jh in range(2):
                        nc.tensor.matmul(
                            cross[:, jh * 512:(jh + 1) * 512],
                            y1T[:, kt * P:(kt + 1) * P],
                            rhs[:, jh * 512:(jh + 1) * 512],
                            start=(kt == 0), stop=(kt == KT - 1))

                # ---- epilogue --------------------------------------------
                diff = ep.tile([P, B], F32, tag="diff")
                nc.vector.tensor_scalar(out=diff[:], in0=cross[:],
                                        scalar1=float(-2.0 / M),
                                        scalar2=None, op0=A.mult)
                nc.vector.tensor_scalar(out=diff[:], in0=diff[:],
                                        scalar1=sqa["y1"][:, 0:1],
                                        scalar2=0.0, op0=A.add, op1=A.max)
                base = ep.tile([P, 1], F32, tag="base")
                nc.vector.tensor_tensor(out=base[:], in0=sqa["y1"][:],
                                        in1=sqa["y2"][:], op=A.add)
                nc.vector.tensor_scalar(out=base[:], in0=base[:], scalar1=1e-8,
                                        scalar2=None, op0=A.add)
                rbase = ep.tile([P, 1], F32, tag="rbase")
                nc.vector.reciprocal(rbase[:], base[:])
                nc.vector.scalar_tensor_tensor(out=diff[:], in0=diff[:],
                                               scalar=2.0,
                                               in1=rbase[:].to_broadcast([P, B]),
                                               op0=A.mult, op1=A.mult)
                lout = ep.tile([P, B], F32, tag="lout")
                nc.scalar.activation(lout[:], diff[:], AF.Sqrt)
                nc.sync.dma_start(out=o_out[:], in_=lout[:])

    nc.compile()
    return nc


def _host_prep(x):
    xmin = np.float32(x[:, 0].min())
    xmax = np.float32(x[:, -1].max())
    grid = np.linspace(np.float32(0.0), np.float32(1.0), M, dtype=np.float32)
    xc = (xmin + grid * (xmax - xmin)).astype(np.float32)[None, :]
    return xmin, xmax, xc


def kernel(x, y1, y2, debug=False, trace=False):
    import ml_dtypes
    from concourse.bass_utils import run_bass_kernel_spmd

    x = np.ascontiguousarray(x, dtype=np.float32)
    y1 = np.ascontiguousarray(y1, dtype=np.float32)
    y2 = np.ascontiguousarray(y2, dtype=np.float32)
    xmin, xmax, xc = _host_prep(x)
    ident = np.eye(P, dtype=ml_dtypes.bfloat16)

    nc = build_nc(float(xmin), float(xmax), debug=debug)
    in_maps = []
    for r in range(NCORES):
        rows = slice(r * R, (r + 1) * R)
        in_maps.append({"x": x[rows], "y1": y1[rows], "y2": y2[rows],
                        "xc": xc, "ident": ident})
    res = run_bass_kernel_spmd(nc, in_maps, core_ids=list(range(NCORES)),
                               trace=trace)
    out = np.concatenate([res.results[r]["out"] for r in range(NCORES)], axis=0)
    if debug or trace:
        return out, res
    return out


# revision 8
# speedup vs baseline: 2.0916x; 1.0994x over previous
# Bass/Trainium2 kernel for nn_L2PairwiceObjectiveFunction (pairwise L2 loss
# between per-row linear interpolations of two curve sets onto a common
# uniform grid).
#
# Full inputs: x, y1, y2 [1024, 8192] f32 (x sorted per row).
# Output: [1024, 1024] f32.
#
# Sharding: batch rows split across 8 NeuronCores (128 rows each, rows on
# SBUF partitions). The pairwise bilinear form uses a chunked AllGather of
# the transposed interpolated y2 grids (bf16) overlapped with second-half
# interpolation, followed by a local PE matmul.
#
# Interpolation: the common grid is UNIFORM, so each data point's grid cell
# is computable elementwise: c[j] = floor((x[j]-xmin)/dx) + 1. For grid
# point m the bracketing segment is the last j with c[j] <= m. Per segment
# the interpolant is linear in g: y(g_m) = A' + Bd*(m - c_j) where
# A' = y_j + B*(g_{c_j} - x_j) (value at the segment's own grid point),
# Bd = B*dx, B = dy/(gap+1e-9). We scatter int16-quantized (A', Bd) for y1
# and y2 (4 arrays) into grid bins with gpsimd local_scatter (per-partition
# indices; last-datum-per-bin dedup keeps indices unique), fill empty bins
# with a carry-forward tensor_tensor_scan, recover (m - c_j) as the carry
# "age" via a second scan form, then interpolate elementwise. Bin space is
# processed in two scatter halves x two scan/interp quarters to fit SBUF.
# Explicit deps keep DVE work out of LocalScatter windows (SBUF contention
# slows concurrent DVE ops ~10x).

import numpy as np

B, N, M, NCORES = 1024, 8192, 3000, 8
R = B // NCORES  # 128 rows per core
P = 128
NBINS = 3004        # bins (c in [0, 3001])
HBINS = 1502        # bins per scatter half: [0,1502), [1502,3004)
QBINS = 751         # bins per scan/interp quarter
NIDX = 4608         # datum window per half (covers Binomial spread at ~11 sigma)
WOFF = (0, N - NIDX)   # window starts per half
CPAD = N + 16       # padded cell-array width (need N+1 for shifted reads)
SA = 1489.0         # A' quantization scale (|A'| <= ~5.6 -> 8.3k of 16383)
AOFF = 16384.0      # A' offset so filled bins are nonzero (empty marker = 0)
SB = 2978.0         # Bd quantization scale (|Bd| clamped to 32725/SB ~ 10.99)
BCLIP = 32725.0
E6 = 1e6            # gap prescale so reciprocal input is ~[1e-3, 2e3]
KT = 24             # matmul k-tiles; grid padded 3000 -> 3072
KT1 = 11            # k-tiles coverable from bin half 0 (cols < 1408 <= 1501)
MT = KT * P
WB = 1024           # stage-A column block
NB = N // WB


def build_nc(xmin, xmax, debug=False):
    import concourse.bacc as bacc
    import concourse.mybir as mybir
    from concourse.tile import TileContext
    from concourse import library_config
    from concourse.tile_rust import add_dep_helper

    F32, BF16, I16 = mybir.dt.float32, mybir.dt.bfloat16, mybir.dt.int16
    A = mybir.AluOpType
    AF = mybir.ActivationFunctionType

    dx = float((np.float32(xmax) - np.float32(xmin)) / np.float32(M - 1))
    inv_dx = float(np.float32(1.0) / np.float32(dx))

    nc = bacc.Bacc("TRN2", target_bir_lowering=False)
    x_in = nc.dram_tensor("x", [R, N], F32, kind="ExternalInput")
    y1_in = nc.dram_tensor("y1", [R, N], F32, kind="ExternalInput")
    y2_in = nc.dram_tensor("y2", [R, N], F32, kind="ExternalInput")
    xc_in = nc.dram_tensor("xc", [1, M], F32, kind="ExternalInput")
    id_in = nc.dram_tensor("ident", [P, P], BF16, kind="ExternalInput")
    o_out = nc.dram_tensor("out", [R, B], F32, kind="ExternalOutput")
    dbg = {}
    if debug:
        for nm, w, dt in [
            ("d_cp1", CPAD, I16), ("d_a1q", N, I16), ("d_b1q", N, I16),
            ("d_fa1", NBINS, F32), ("d_fb1", NBINS, F32),
            ("d_age", NBINS, F32),
            ("d_y1c", M, BF16), ("d_y2c", M, BF16), ("d_sq1", 1, F32),
        ]:
            dbg[nm] = nc.dram_tensor(nm, [R, w], dt, kind="ExternalOutput")

    with TileContext(nc) as tc:
        with (
            tc.tile_pool(name="pers", bufs=1) as pers,
            tc.tile_pool(name="psum", bufs=2, space="PSUM") as pp,
            tc.tile_pool(name="mmpsum", bufs=1, space="PSUM") as mmpp,
            tc.tile_pool(name="dram", bufs=1, space="DRAM") as dp,
        ):
            lib_bi = nc.gpsimd.load_library(library_config.local_scatter)

            x0 = pers.tile([P, 1], F32, tag="x0")
            xlast = pers.tile([P, 1], F32, tag="xlast")
            nc.sync.dma_start(out=x0[:], in_=x_in[:, 0:1])
            nc.sync.dma_start(out=xlast[:], in_=x_in[:, N - 1:N])

            y1c = pers.tile([P, MT], BF16, tag="y1c")
            y2c = pers.tile([P, MT], BF16, tag="y2c")
            nc.vector.memset(y1c[:, M:], 0)
            nc.vector.memset(y2c[:, M:], 0)
            ident = pers.tile([P, P], BF16, tag="ident")
            nc.sync.dma_start(out=ident[:], in_=id_in[:])
            y1T = pers.tile([P, MT], BF16, tag="y1T")
            y2T = pers.tile([P, MT], BF16, tag="y2T")
            sqacc = {}
            for ynm in ("y1", "y2"):
                s = pers.tile([P, 1], F32, tag=f"sqacc_{ynm}")
                nc.vector.memset(s[:], 0)
                sqacc[ynm] = s
            carries = {}   # latest scan carry [P,1] per array
            inits = {}     # scan initials from first datum
            ANAMES = ("age", "a1", "b1", "a2", "b2")
            cross = mmpp.tile([P, B], F32, space="PSUM")

            def transpose_tiles(k_lo, k_hi):
                for kt in range(k_lo, k_hi):
                    for src, dstt in [(y1c, y1T), (y2c, y2T)]:
                        ps = pp.tile([P, P], BF16, tag="tps", space="PSUM")
                        nc.tensor.transpose(out=ps[:],
                                            in_=src[:, kt * P:(kt + 1) * P],
                                            identity=ident[:])
                        nc.vector.tensor_copy(out=dstt[:, kt * P:(kt + 1) * P],
                                              in_=ps[:])

            def mm_chunk(rpool, agout, k_lo, k_hi):
                agv = agout[:].rearrange("(r p) f -> r p f", r=NCORES)
                for kt in range(k_lo, k_hi):
                    rhs = rpool.tile([P, B], BF16, tag="rhs")
                    lk = (kt - k_lo) * P
                    nc.sync.dma_start(
                        out=rhs[:].rearrange("p (r f) -> p r f", r=NCORES),
                        in_=agv[:, :, lk:lk + P].rearrange("r p f -> p r f"))
                    for jh in range(2):
                        nc.tensor.matmul(
                            cross[:, jh * 512:(jh + 1) * 512],
                            y1T[:, kt * P:(kt + 1) * P],
                            rhs[:, jh * 512:(jh + 1) * 512],
                            start=(kt == 0), stop=(kt == KT - 1))

            with (
                tc.tile_pool(name="qp", bufs=1) as qp,
                tc.tile_pool(name="ldp", bufs=2) as ldp,
                tc.tile_pool(name="sp", bufs=1) as sp,
                tc.tile_pool(name="rhsp", bufs=3) as rhsp,
            ):
                # ---- stage A: per-datum cells + interp coefficients ------
                cp1 = qp.tile([P, CPAD], I16, tag="cp1")   # c[j] + 1
                a1q = qp.tile([P, N], I16, tag="a1q")
                b1q = qp.tile([P, N], I16, tag="b1q")
                a2q = qp.tile([P, N], I16, tag="a2q")
                b2q = qp.tile([P, N], I16, tag="b2q")
                nc.vector.memset(cp1[:, N:], 30000)
                QARR = {"a1": a1q, "b1": b1q, "a2": a2q, "b2": b2q}
                lastA = None

                for bi in range(NB):
                    lo = bi * WB
                    wext = WB + 1 if lo + WB < N else WB
                    sl = slice(lo, lo + WB)
                    xb = ldp.tile([P, WB + 1], F32, tag="xb")
                    yb1 = ldp.tile([P, WB + 1], F32, tag="yb1")
                    yb2 = ldp.tile([P, WB + 1], F32, tag="yb2")
                    nc.sync.dma_start(out=xb[:, :wext], in_=x_in[:, lo:lo + wext])
                    nc.sync.dma_start(out=yb1[:, :wext], in_=y1_in[:, lo:lo + wext])
                    nc.sync.dma_start(out=yb2[:, :wext], in_=y2_in[:, lo:lo + wext])
                    if wext == WB:
                        nc.vector.memset(xb[:, WB:], 0)
                        nc.vector.memset(yb1[:, WB:], 0)
                        nc.vector.memset(yb2[:, WB:], 0)
                    # t5 = (x - xmin)/dx + 1.5  (in [1.5, 3000.5] for real x)
                    t5 = sp.tile([P, WB], F32, tag="t5")
                    nc.scalar.activation(t5[:], xb[:, :WB], AF.Copy,
                                         bias=float(1.5 - xmin * inv_dx),
                                         scale=inv_dx)
                    # cp1 = round(t5) = c + 1
                    nc.vector.tensor_copy(out=cp1[:, sl], in_=t5[:])
                    cf = sp.tile([P, WB], F32, tag="cf")
                    nc.scalar.copy(out=cf[:], in_=cp1[:, sl])
                    gd = sp.tile([P, WB], F32, tag="gd")
                    nc.vector.tensor_tensor(out=gd[:], in0=cf[:], in1=t5[:],
                                            op=A.subtract)
                    # gx = (g_c - x) * 1e6 = (gd + 0.5) * dx * 1e6
                    gx = sp.tile([P, WB], F32, tag="t5")  # t5 dead
                    nc.scalar.activation(gx[:], gd[:], AF.Copy,
                                         bias=float(0.5 * dx * E6),
                                         scale=float(dx * E6))
                    gap = sp.tile([P, WB], F32, tag="gap")
                    nc.vector.tensor_tensor(out=gap[:], in0=xb[:, 1:WB + 1],
                                            in1=xb[:, :WB], op=A.subtract)
                    # r6 = 1/(gap*1e6 + 1e-3) = recip(gap + 1e-9) * 1e-6
                    gape = sp.tile([P, WB], F32, tag="cf")  # cf dead
                    nc.scalar.activation(gape[:], gap[:], AF.Copy,
                                         bias=1e-3, scale=float(E6))
                    r6 = sp.tile([P, WB], F32, tag="r6")
                    nc.vector.reciprocal_approx_fast(out=r6[:], in_=gape[:])
                    for ynm, yb, aq, bq in (("1", yb1, a1q, b1q),
                                            ("2", yb2, a2q, b2q)):
                        dy = sp.tile([P, WB], F32, tag="gap")  # gap dead
                        nc.vector.tensor_tensor(out=dy[:], in0=yb[:, 1:WB + 1],
                                                in1=yb[:, :WB], op=A.subtract)
                        bb = sp.tile([P, WB], F32, tag="gd")  # gd dead
                        nc.vector.tensor_tensor(out=bb[:], in0=dy[:], in1=r6[:],
                                                op=A.mult)
                        # A' = y + B*gx
                        t6 = sp.tile([P, WB], F32, tag="t6")
                        nc.vector.tensor_tensor(out=t6[:], in0=bb[:], in1=gx[:],
                                                op=A.mult)
                        nc.vector.tensor_tensor(out=t6[:], in0=t6[:],
                                                in1=yb[:, :WB], op=A.add)
                        nc.scalar.activation(aq[:, sl], t6[:], AF.Copy,
                                             bias=AOFF, scale=SA)
                        # Bd*SB = B*dx*SB, clamped to +-BCLIP
                        bt = sp.tile([P, WB], F32, tag="gap")  # dy dead
                        nc.vector.tensor_scalar(out=bt[:], in0=bb[:],
                                                scalar1=float(E6 * dx * SB),
                                                scalar2=BCLIP,
                                                op0=A.mult, op1=A.min)
                        lastA = nc.vector.tensor_scalar(
                            out=bq[:, sl], in0=bt[:], scalar1=-BCLIP,
                            scalar2=None, op0=A.max)

                # scan initials from datum 0 (flat extension: B = 0)
                for nm, src in [("a1", a1q[:, 0:1]), ("a2", a2q[:, 0:1])]:
                    it = pers.tile([P, 1], F32, tag=f"init_{nm}")
                    nc.vector.tensor_copy(out=it[:], in_=src)
                    inits[nm] = it
                inits["b1"] = 0.0
                inits["b2"] = 0.0
                inits["age"] = 0.0

                if debug:
                    nc.sync.dma_start(out=dbg["d_cp1"][:], in_=cp1[:])
                    nc.sync.dma_start(out=dbg["d_a1q"][:], in_=a1q[:])
                    nc.sync.dma_start(out=dbg["d_b1q"][:], in_=b1q[:])

                last_sc = None   # last scatter of previous half
                last_interp = None
                for h in range(2):
                    woff = WOFF[h]
                    # ---- dedup + scatter index for this half -------------
                    neq = qp.tile([P, NIDX], I16, tag="neq")
                    neq_bi = nc.vector.tensor_tensor(
                        out=neq[:], in0=cp1[:, woff:woff + NIDX],
                        in1=cp1[:, woff + 1:woff + NIDX + 1], op=A.not_equal)
                    if h == 1:
                        # keep DVE out of the h0 scatter window
                        add_dep_helper(neq_bi.ins, last_sc.ins, sync=True,
                                       reason="dedup after scatters")
                        # second-to-last datum always survives; last never
                        nc.vector.memset(neq[:, NIDX - 2:NIDX - 1], 1)
                        nc.vector.memset(neq[:, NIDX - 1:], 0)
                    ix = qp.tile([P, NIDX], I16, tag="idx")
                    nc.vector.tensor_tensor(out=ix[:], in0=neq[:],
                                            in1=cp1[:, woff:woff + NIDX],
                                            op=A.mult)
                    if h == 0:
                        # bin = c = cp1 - 1; bins > 1501 pushed negative
                        nc.vector.tensor_scalar(out=ix[:], in0=ix[:], scalar1=1,
                                                scalar2=None, op0=A.subtract)
                        sel = qp.tile([P, NIDX], I16, tag="neq")  # reuse slot
                        nc.vector.tensor_scalar(out=sel[:], in0=ix[:],
                                                scalar1=HBINS - 1, scalar2=None,
                                                op0=A.is_gt)
                        nc.vector.scalar_tensor_tensor(out=ix[:], in0=sel[:],
                                                       scalar=-32000.0,
                                                       in1=ix[:], op0=A.mult,
                                                       op1=A.add)
                    else:
                        # bin = c - 1502; bins < 1502 go negative (ignored)
                        nc.vector.tensor_scalar(out=ix[:], in0=ix[:],
                                                scalar1=HBINS + 1, scalar2=None,
                                                op0=A.subtract)

                    # ---- 4 scatters --------------------------------------
                    dsts = {}
                    for si, nm in enumerate(("a1", "b1", "a2", "b2")):
                        dst = qp.tile([P, HBINS], I16, tag=f"dst_{nm}")
                        sc_bi = nc.gpsimd.local_scatter(
                            dst[:], QARR[nm][:, woff:woff + NIDX], ix[:],
                            channels=P, num_elems=HBINS, num_idxs=NIDX)
                        add_dep_helper(sc_bi.ins, lib_bi.ins, sync=True,
                                       reason="lib before scatter")
                        if si == 0:
                            # no scatter before all coefficients are final
                            add_dep_helper(sc_bi.ins, lastA.ins, sync=True,
                                           reason="stage A before scatters")
                            if last_interp is not None:
                                add_dep_helper(sc_bi.ins, last_interp.ins,
                                               sync=True,
                                               reason="h0 interp before h1 scat")
                        dsts[nm] = dst
                        last_sc = sc_bi

                    # ---- per quarter: fill scans + interpolation ---------
                    for qh in range(2):
                        qb0 = h * HBINS + qh * QBINS
                        qs = slice(qh * QBINS, (qh + 1) * QBINS)
                        first = (h == 0 and qh == 0)
                        emt = sp.tile([P, WB], F32, tag="t5")
                        emt_bi = nc.vector.tensor_scalar(
                            out=emt[:, :QBINS], in0=dsts["a1"][:, qs],
                            scalar1=0, scalar2=None, op0=A.is_equal)
                        if qh == 0:
                            # scans run only after the scatter window closes
                            add_dep_helper(emt_bi.ins, last_sc.ins, sync=True,
                                           reason="scans after scatters")
                        filled = {}
                        FTAG = {"age": "cf", "a1": "gd", "b1": "gap",
                                "a2": "r6", "b2": "t6"}
                        for nm in ANAMES:
                            f = sp.tile([P, WB], F32, tag=FTAG[nm])
                            init = inits[nm] if first else carries[nm]
                            init_ap = (init if isinstance(init, float)
                                       else init[:, 0:1])
                            src = (emt[:, :QBINS] if nm == "age"
                                   else dsts[nm][:, qs])
                            nc.vector.tensor_tensor_scan(
                                f[:, :QBINS], emt[:, :QBINS], src, init_ap,
                                A.mult, A.add)
                            filled[nm] = f
                            cy = pers.tile([P, 1], F32, tag=f"carry_{nm}")
                            nc.vector.tensor_copy(
                                out=cy[:], in_=f[:, QBINS - 1:QBINS])
                            carries[nm] = cy

                        if debug:
                            for dnm, key in [("d_fa1", "a1"), ("d_fb1", "b1"),
                                             ("d_age", "age")]:
                                nc.sync.dma_start(
                                    out=dbg[dnm][:, qb0:qb0 + QBINS],
                                    in_=filled[key][:, :QBINS])

                        # interpolation over grid m in [qb0, min(qb0+QBINS, M))
                        W = min(qb0 + QBINS, M) - qb0
                        if W <= 0:
                            continue
                        fsl = slice(0, W)
                        xcb = sp.tile([P, WB], F32, tag="q7")
                        nc.sync.dma_start(
                            out=xcb[:, :W],
                            in_=xc_in[:, qb0:qb0 + W].to_broadcast([P, W]))
                        ma = sp.tile([P, WB], F32, tag="q8")
                        nc.vector.tensor_scalar(out=ma[:, :W], in0=xcb[:, :W],
                                                scalar1=x0[:, 0:1], scalar2=None,
                                                op0=A.is_ge)
                        scr1 = sp.tile([P, WB], F32, tag="q9")
                        nc.vector.tensor_scalar(out=scr1[:, :W], in0=xcb[:, :W],
                                                scalar1=xlast[:, 0:1],
                                                scalar2=None, op0=A.is_le)
                        nc.vector.tensor_tensor(out=ma[:, :W], in0=ma[:, :W],
                                                in1=scr1[:, :W], op=A.mult)
                        for ynm, yc in (("1", y1c), ("2", y2c)):
                            # y = (fa - AOFF)/SA + (fb/SB)*age
                            t1 = sp.tile([P, WB], F32, tag="q9")
                            nc.vector.scalar_tensor_tensor(
                                out=t1[:, :W], in0=filled["b" + ynm][:, fsl],
                                scalar=float(1.0 / SB),
                                in1=filled["age"][:, fsl],
                                op0=A.mult, op1=A.mult)
                            t2 = sp.tile([P, WB], F32, tag="q7")
                            nc.vector.tensor_scalar(
                                out=t2[:, :W], in0=filled["a" + ynm][:, fsl],
                                scalar1=-AOFF, scalar2=float(1.0 / SA),
                                op0=A.add, op1=A.mult)
                            nc.vector.tensor_tensor(out=t2[:, :W],
                                                    in0=t2[:, :W],
                                                    in1=t1[:, :W], op=A.add)
                            nc.vector.tensor_tensor(out=yc[:, qb0:qb0 + W],
                                                    in0=t2[:, :W],
                                                    in1=ma[:, :W], op=A.mult)
                            spt = qp.tile([P, 1], F32, tag="spt")
                            e2 = sp.tile([P, WB], F32, tag="q9")
                            nc.scalar.activation(e2[:, :W], yc[:, qb0:qb0 + W],
                                                 AF.Square,
                                                 accum_out=spt[:, 0:1])
                            last_interp = nc.vector.tensor_tensor(
                                out=sqacc["y" + ynm][:],
                                in0=sqacc["y" + ynm][:],
                                in1=spt[:], op=A.add)

                    if h == 0:
                        # ---- overlap: transpose + AG + matmul of chunk 1 -
                        transpose_tiles(0, KT1)
                        agin1 = dp.tile([P, KT1 * P], BF16)
                        agout1 = dp.tile([NCORES * P, KT1 * P], BF16,
                                         addr_space="Shared")
                        nc.sync.dma_start(out=agin1[:], in_=y2T[:, 0:KT1 * P])
                        nc.gpsimd.collective_compute(
                            "AllGather", A.bypass,
                            replica_groups=[list(range(NCORES))],
                            ins=[agin1[:].opt()], outs=[agout1[:].opt()])
                        mm_chunk(rhsp, agout1, 0, KT1)

            # ---- sq = mean(y^2) ------------------------------------------
            sqa = {}
            for ynm in ("y1", "y2"):
                s = pers.tile([P, 1], F32, tag=f"sqa_{ynm}")
                nc.vector.tensor_scalar(out=s[:], in0=sqacc[ynm][:],
                                        scalar1=float(1.0 / M), scalar2=None,
                                        op0=A.mult)
                sqa[ynm] = s

            if debug:
                nc.sync.dma_start(out=dbg["d_y1c"][:], in_=y1c[:, 0:M])
                nc.sync.dma_start(out=dbg["d_y2c"][:], in_=y2c[:, 0:M])
                nc.sync.dma_start(out=dbg["d_sq1"][:], in_=sqa["y1"][:])

            with (
                tc.tile_pool(name="ep", bufs=1) as ep,
                tc.tile_pool(name="rhsp2", bufs=3) as rhsp2,
            ):
                # ---- transposes chunk 2 ----------------------------------
                transpose_tiles(KT1, KT)

                # ---- fold -1500*sq2[j] into spare matmul k-slots ---------
                # Slots m=3008,3009 (partition 64 of the last k-tile;
                # vector ops need partition base 0/32/64/96) carry
                # bf16 hi/res halves of v = -(M/2)*sq2; with lhs slots = 1,
                # cross picks up -1500*sq2[j], and (-2/M)*cross then
                # includes +sq2[j] exactly -- no [B]-wide broadcast needed.
                sq2q = ep.tile([P, 2], BF16, tag="sq2q")
                nc.vector.tensor_scalar(out=sq2q[:, 0:1], in0=sqa["y2"][:],
                                        scalar1=float(-M / 2.0), scalar2=None,
                                        op0=A.mult)
                qhf = ep.tile([P, 1], F32, tag="qhf")
                nc.scalar.copy(out=qhf[:], in_=sq2q[:, 0:1])
                nc.vector.scalar_tensor_tensor(out=sq2q[:, 1:2],
                                               in0=sqa["y2"][:],
                                               scalar=float(-M / 2.0),
                                               in1=qhf[:], op0=A.mult,
                                               op1=A.subtract)
                scrd = dp.tile([P, 2], BF16)
                nc.sync.dma_start(out=scrd[:], in_=sq2q[:])
                nc.sync.dma_start(out=y2T[64:66, (KT - 1) * P:KT * P],
                                  in_=scrd[:].rearrange("i q -> q i"))
                nc.vector.memset(y1T[64:66, (KT - 1) * P:KT * P], 1.0)

                # ---- AllGather chunk 2 (bf16) ----------------------------
                agin2 = dp.tile([P, (KT - KT1) * P], BF16)
                agout2 = dp.tile([NCORES * P, (KT - KT1) * P], BF16,
                                 addr_space="Shared")
                nc.sync.dma_start(out=agin2[:], in_=y2T[:, KT1 * P:MT])
                nc.gpsimd.collective_compute(
                    "AllGather", A.bypass,
                    replica_groups=[list(range(NCORES))],
                    ins=[agin2[:].opt()], outs=[agout2[:].opt()])

                # ---- matmul chunk 2 --------------------------------------
                mm_chunk(rhsp2, agout2, KT1, KT)

                # ---- epilogue --------------------------------------------
                diff = ep.tile([P, B], F32, tag="diff")
                nc.vector.tensor_scalar(out=diff[:], in0=cross[:],
                                        scalar1=float(-2.0 / M),
                                        scalar2=None, op0=A.mult)
                nc.vector.tensor_scalar(out=diff[:], in0=diff[:],
                                        scalar1=sqa["y1"][:, 0:1],
                                        scalar2=0.0, op0=A.add, op1=A.max)
                base = ep.tile([P, 1], F32, tag="base")
                nc.vector.tensor_tensor(out=base[:], in0=sqa["y1"][:],
                                        in1=sqa["y2"][:], op=A.add)
                nc.vector.tensor_scalar(out=base[:], in0=base[:], scalar1=1e-8,
                                        scalar2=None, op0=A.add)
                rbase = ep.tile([P, 1], F32, tag="rbase")
                nc.vector.reciprocal(rbase[:], base[:])
                nc.vector.scalar_tensor_tensor(out=diff[:], in0=diff[:],
                                               scalar=2.0,
                                               in1=rbase[:].to_broadcast([P, B]),
                                               op0=A.mult, op1=A.mult)
                lout = ep.tile([P, B], F32, tag="lout")
                nc.scalar.activation(lout[:], diff[:], AF.Sqrt)
                nc.sync.dma_start(out=o_out[:], in_=lout[:])

    nc.compile()
    return nc


def _host_prep(x):
    xmin = np.float32(x[:, 0].min())
    xmax = np.float32(x[:, -1].max())
    grid = np.linspace(np.float32(0.0), np.float32(1.0), M, dtype=np.float32)
    xc = (xmin + grid * (xmax - xmin)).astype(np.float32)[None, :]
    return xmin, xmax, xc


def kernel(x, y1, y2, debug=False, trace=False):
    import ml_dtypes
    from concourse.bass_utils import run_bass_kernel_spmd

    x = np.ascontiguousarray(x, dtype=np.float32)
    y1 = np.ascontiguousarray(y1, dtype=np.float32)
    y2 = np.ascontiguousarray(y2, dtype=np.float32)
    xmin, xmax, xc = _host_prep(x)
    ident = np.eye(P, dtype=ml_dtypes.bfloat16)

    nc = build_nc(float(xmin), float(xmax), debug=debug)
    in_maps = []
    for r in range(NCORES):
        rows = slice(r * R, (r + 1) * R)
        in_maps.append({"x": x[rows], "y1": y1[rows], "y2": y2[rows],
                        "xc": xc, "ident": ident})
    res = run_bass_kernel_spmd(nc, in_maps, core_ids=list(range(NCORES)),
                               trace=trace)
    out = np.concatenate([res.results[r]["out"] for r in range(NCORES)], axis=0)
    if debug or trace:
        return out, res
    return out
